# revision 23
# baseline (speedup 1.0000x reference)
"""Trainium2 Bass kernel for nn_MCPBRNN_Generic_PETconstraint_Scaling_BYPASSM1.

Algorithm
---------
The module is a 1M-step scalar (H=1) nonlinear recurrence
    c' = c - oo1*sig(A_OO*c+E_OO)*c - min(ol_t*c, u2_t) + (1-sig(A_IB*c+D_t))*u1_t
followed by 12 elementwise outputs.  The recurrence is strongly contracting
(|dc'/dc| in [0.37, 0.69] measured along the true trajectory; worst 24-step
window product ~3e-7), so the time axis can be chunked: each chunk starts from
c=0 and runs W warm-up steps over the preceding inputs, after which its
state agrees with the true trajectory to below f32 resolution.

Sharding: time axis split across 8 cores (125000 steps each); within a core,
128 partitions x K=64 slots = 8192 chunks of L=16 steps stepped in SIMD
lock-step.  Per step the work is split across engines: 2 sigmoids on ACT,
4 TT + 1 TS on DVE, and the min-branch (3 ops) on GpSimd.  The c history is
then re-laid out to linear time order with one strided tensor_copy and all
outputs are computed in bulk elementwise passes (ACT affine copies + DVE +
GpSimd) and DMA'd out contiguously.  obsstd = std(y_obs[1000:800000], ddof=1)
is accumulated by ACT Copy/Square accum_out chunks interleaved into the
scan's ACT idle slots, then finished with two PE matmuls and a Newton rsqrt.
"""
import math
import numpy as np
from contextlib import ExitStack

import concourse.bass as bass
import concourse.mybir as mybir
import concourse.tile as tile
from concourse.bass_utils import run_bass_kernel_spmd

f32 = np.float32

# ---- problem constants (hardcoded from the module definition) ----
B = 1000000
SPIN, TRAIN = 1000, 800000
ML, SL, U1_MAX = 2.9086, 1.898, 221.519

# ---- sharding geometry ----
NCORE = 8
P = 128          # SBUF partitions
K = 64           # chunk slots per partition
L = 16           # real steps per chunk
W = 24           # warm-up steps per chunk
S = W + L        # total scan steps (40)
F = K * L        # linear free size per partition (1024)
PAD = P * F      # padded per-core length (131072)
PER_CORE = B // NCORE   # 125000
NYO = TRAIN - SPIN      # 799000
YF = 6272               # yobs free size: 128*6272 = 802816 >= NYO
NOBS = 16               # obs accumulation chunks per statistic
YCH = YF // NOBS        # 392 elements per chunk
OBS_START = 12          # first scan step after which obs chunks interleave

DBG_SKIP = set()        # debug: subsets of {"scan", "phased", "obs", "outdma"}
SCAN_VARIANT = "pool_m"  # "dve_all" | "pool_m" | "pool_mp"

DT = mybir.dt.float32
AF = mybir.ActivationFunctionType
OP = mybir.AluOpType


def legalize_waits(nc, max_waits=1):
    """This toolchain's walrus accepts only one sync-wait per compute
    instruction.  Hoist extra waits onto same-engine NOPs inserted right
    before the gated instruction (engine queues execute in order, so the
    semantics are identical)."""
    eng_map = {
        mybir.EngineType.DVE: nc.vector,
        mybir.EngineType.Activation: nc.scalar,
        mybir.EngineType.Pool: nc.gpsimd,
        mybir.EngineType.PE: nc.tensor,
        mybir.EngineType.SP: nc.sync,
    }
    blocks = nc.m.functions[0].blocks

    def detach(ins_obj):
        for bb2 in blocks:
            try:
                bb2.instructions.remove(ins_obj)
                return
            except ValueError:
                continue

    for bb in blocks:
        i = 0
        while i < len(bb.instructions):
            inst = bb.instructions[i]
            si = getattr(inst, "sync_info", None)
            if si is not None and si.on_wait and len(si.on_wait) > max_waits \
                    and inst.engine in eng_map:
                waits = list(si.on_wait)
                keep, extras = waits[-max_waits:], waits[:-max_waits]
                inst.sync_info = mybir.SyncInfo(on_wait=keep, on_update=list(si.on_update))
                e = eng_map[inst.engine]
                for w in extras:
                    nop = e.nop().ins
                    detach(nop)
                    nop.sync_info = mybir.SyncInfo(on_wait=[w], on_update=[])
                    bb.instructions.insert(i, nop)
                    i += 1
            i += 1


def build_program(c):
    """c: dict of baked float constants."""
    nc = bass.Bass()
    u1t = nc.declare_dram_parameter("u1t", [P, S * K], DT, isOutput=False)
    u2t = nc.declare_dram_parameter("u2t", [P, S * K], DT, isOutput=False)
    u1l = nc.declare_dram_parameter("u1l", [P, F], DT, isOutput=False)
    u2l = nc.declare_dram_parameter("u2l", [P, F], DT, isOutput=False)
    yob = nc.declare_dram_parameter("yob", [P, YF], DT, isOutput=False)
    outs = {}
    for nm in ["o_h", "o_c", "o_l", "o_lc", "o_bp", "o_ib", "o_oo", "o_ol", "o_olc", "o_f", "o_obs"]:
        outs[nm] = nc.declare_dram_parameter(nm, [P, F], DT, isOutput=True)
    outs["o_hio"] = nc.declare_dram_parameter("o_hio", [P, 2 * F], DT, isOutput=True)

    OBS = "obs" not in DBG_SKIP
    NSTEP = S if "scan" not in DBG_SKIP else 1
    PHD = "phased" not in DBG_SKIP

    with tile.TileContext(nc) as tc, ExitStack() as ctx:
        pers = ctx.enter_context(tc.tile_pool(name="pers", bufs=1))
        scr = ctx.enter_context(tc.tile_pool(name="scr", bufs=4))
        pd = ctx.enter_context(tc.tile_pool(name="pd", bufs=1))
        pp = ctx.enter_context(tc.tile_pool(name="psum", bufs=1, space="PSUM"))

        # ---------- load inputs (scan inputs in step-chunks for early start) ----------
        NCH = 5
        CHB = [0, 4, 10, 18, 28, S]            # step boundaries per chunk
        u1t_t = pers.tile([P, S * K], DT, tag="u1t_t")
        u2t_t = pers.tile([P, S * K], DT, tag="u2t_t")
        for q in range(NCH):
            lo, hi = CHB[q] * K, CHB[q + 1] * K
            nc.sync.dma_start(u1t_t[:, lo:hi], u1t[:, lo:hi])
            nc.sync.dma_start(u2t_t[:, lo:hi], u2t[:, lo:hi])
        u1l_t = pers.tile([P, F], DT, tag="u1l_t")
        u2l_t = pers.tile([P, F], DT, tag="u2l_t")
        nc.sync.dma_start(u1l_t[:], u1l[:])
        nc.sync.dma_start(u2l_t[:], u2l[:])
        yt = pers.tile([P, YF], DT, tag="yt")
        if OBS:
            nc.sync.dma_start(yt[:], yob[:])

        # bias tiles for activations (const-AP pool only has 0.0/1.0)
        b_bol = pers.tile([P, 1], DT, tag="b_bol")
        nc.vector.memset(b_bol[:], c["B_OL"])
        b_eoo = pers.tile([P, 1], DT, tag="b_eoo")
        nc.vector.memset(b_eoo[:], c["E_OO"])

        # ---------- scan input prep (same chunking as the DMAs) ----------
        # Dp = D0 + D1*u1   (ib = sig(A_IB*(c + Dp)))
        dpt = pers.tile([P, S * K], DT, tag="dpt")
        olt = pers.tile([P, S * K], DT, tag="olt")
        for q in range(NCH):
            lo, hi = CHB[q] * K, CHB[q + 1] * K
            nc.vector.tensor_scalar(dpt[:, lo:hi], u1t_t[:, lo:hi], c["D1"], c["D0"], OP.mult, OP.add)
            # ol = OL1 * sig(SU2*u2 + B_OL)
            nc.scalar.activation(olt[:, lo:hi], u2t_t[:, lo:hi], AF.Sigmoid, bias=b_bol[:], scale=c["SU2"])
            nc.vector.tensor_scalar(olt[:, lo:hi], olt[:, lo:hi], c["OL1"], None, OP.mult)

        # obs sums: Sum(y) on idle PE; y^2 via ACT Square chunks interleaved
        # into the scan's ACT idle slots (in-place, after Sum(y) matmuls);
        # then Sum(y^2) on PE again.
        ones128 = pers.tile([P, 1], DT, tag="ones128")
        nc.gpsimd.memset(ones128[:], 1.0)
        ps_sy = pp.tile([P, 1], DT, tag="ps_sy")
        ps_sy2 = pp.tile([P, 1], DT, tag="ps_sy2")
        SQK = YF // 128
        obs_jobs = []
        if OBS:
            for q in range(SQK):
                obs_jobs.append(("my", q))
            for j in range(NOBS):
                obs_jobs.append(("sq", j))
            for q in range(SQK):
                obs_jobs.append(("my2", q))

        def emit_obs_job(job):
            kind, j = job
            if kind == "my":
                nc.tensor.matmul(ps_sy[:], yt[:, j * 128:(j + 1) * 128], ones128[:],
                                 start=(j == 0), stop=(j == SQK - 1))
            elif kind == "sq":
                sl = yt[:, j * YCH:(j + 1) * YCH]
                nc.scalar.activation(sl, sl, AF.Square)
            else:
                nc.tensor.matmul(ps_sy2[:], yt[:, j * 128:(j + 1) * 128], ones128[:],
                                 start=(j == 0), stop=(j == SQK - 1))

        # ---------- the scan ----------
        ch = pers.tile([P, (S + 1) * K], DT, tag="ch")
        chv = ch[:].rearrange("p (s k) -> p s k", k=K)
        nc.vector.memset(chv[:, 0, :], 0.0)
        u1v = u1t_t[:].rearrange("p (s k) -> p s k", k=K)
        u2v = u2t_t[:].rearrange("p (s k) -> p s k", k=K)
        dpv = dpt[:].rearrange("p (s k) -> p s k", k=K)
        olv = olt[:].rearrange("p (s k) -> p s k", k=K)
        obs_i = 0
        for i in range(NSTEP):
            cc = chv[:, i, :]
            # critical chain first: t1 -> a1 -> g_ -> p_ -> c1
            t1 = scr.tile([P, K], DT, tag="t1", name="t1")
            nc.scalar.activation(t1[:], cc, AF.Sigmoid, bias=b_eoo[:], scale=c["A_OO"])
            s_ = scr.tile([P, K], DT, tag="s_", name="s_")
            nc.vector.tensor_tensor(s_[:], cc, dpv[:, i, :], OP.add)
            t2c = scr.tile([P, K], DT, tag="t2c", name="t2c")
            nc.scalar.activation(t2c[:], s_[:], AF.Sigmoid, scale=-c["A_IB"])   # 1-ib
            me = nc.gpsimd if SCAN_VARIANT in ("pool_m", "pool_mp") else nc.vector
            pe_ = nc.gpsimd if SCAN_VARIANT == "pool_mp" else nc.vector
            m_ = scr.tile([P, K], DT, tag="m_", name="m_")
            me.tensor_tensor(m_[:], cc, olv[:, i, :], OP.mult)
            m2 = scr.tile([P, K], DT, tag="m2", name="m2")
            nc.vector.tensor_tensor(m2[:], m_[:], u2v[:, i, :], OP.min)
            a1 = scr.tile([P, K], DT, tag="a1", name="a1")
            nc.vector.tensor_scalar(a1[:], t1[:], -c["OO1"], 1.0, OP.mult, OP.add)
            g_ = scr.tile([P, K], DT, tag="g_", name="g_")
            nc.vector.tensor_tensor(g_[:], cc, a1[:], OP.mult)
            p_ = scr.tile([P, K], DT, tag="p_", name="p_")
            pe_.tensor_tensor(p_[:], g_[:], m2[:], OP.subtract)
            q_ = scr.tile([P, K], DT, tag="q_", name="q_")
            nc.vector.tensor_tensor(q_[:], t2c[:], u1v[:, i, :], OP.mult)
            nc.vector.tensor_tensor(chv[:, i + 1, :], p_[:], q_[:], OP.add)
            if i >= OBS_START:
                budget = 6
                while obs_i < len(obs_jobs) and budget > 0:
                    kind = obs_jobs[obs_i][0]
                    emit_obs_job(obs_jobs[obs_i]); obs_i += 1
                    budget -= 6 if kind == "sq" else 1
        while obs_i < len(obs_jobs):
            emit_obs_job(obs_jobs[obs_i]); obs_i += 1

        # ---------- finish obsstd: cross-partition + newton rsqrt ----------
        sb_sy = pers.tile([P, 1], DT, tag="sb_sy")
        sb_sy2 = pers.tile([P, 1], DT, tag="sb_sy2")
        if OBS:
            nc.vector.tensor_copy(sb_sy[:], ps_sy[:])
            nc.vector.tensor_copy(sb_sy2[:], ps_sy2[:])
        else:
            nc.vector.memset(sb_sy[:], 0.5)
            nc.vector.memset(sb_sy2[:], 0.5)
        ps_t1 = pp.tile([1, 1], DT, tag="ps_t1")
        ps_t2 = pp.tile([1, 1], DT, tag="ps_t2")
        nc.tensor.matmul(ps_t1[:], sb_sy[:], ones128[:], start=True, stop=True)
        nc.tensor.matmul(ps_t2[:], sb_sy2[:], ones128[:], start=True, stop=True)
        s1b = pers.tile([1, 1], DT, tag="s1b")
        s2b = pers.tile([1, 1], DT, tag="s2b")
        nc.vector.tensor_copy(s1b[:], ps_t1[:])
        nc.vector.tensor_copy(s2b[:], ps_t2[:])
        # var = (S2 - S1^2/n) / (n-1)
        va = pers.tile([1, 1], DT, tag="va")
        nc.vector.tensor_tensor(va[:], s1b[:], s1b[:], OP.mult)
        vb = pers.tile([1, 1], DT, tag="vb")
        nc.vector.scalar_tensor_tensor(vb[:], va[:], -1.0 / NYO, s2b[:], OP.mult, OP.add)
        nc.vector.tensor_scalar(vb[:], vb[:], 1.0 / (NYO - 1), None, OP.mult)
        # std = vb * rsqrt(vb) via bit-trick seed + 3 Newton iterations
        vbi = vb[:].bitcast(mybir.dt.int32)
        shr = pers.tile([1, 1], mybir.dt.int32, tag="shr")
        nc.vector.tensor_scalar(shr[:], vbi, 1, None, OP.arith_shift_right)
        kmagic = pers.tile([1, 1], mybir.dt.int32, tag="kmagic")
        nc.vector.memset(kmagic[:], 0x5F3759DF)
        seed = pers.tile([1, 1], mybir.dt.int32, tag="seed")
        nc.vector.tensor_tensor(seed[:], kmagic[:], shr[:], OP.subtract)
        y_ = seed[:].bitcast(mybir.dt.float32)
        for it in range(3):
            t_a = scr.tile([1, 1], DT, tag="nr_a", name="nr_a")
            nc.vector.tensor_tensor(t_a[:], vb[:], y_, OP.mult)       # v*y
            t_b = scr.tile([1, 1], DT, tag="nr_b", name="nr_b")
            nc.vector.tensor_tensor(t_b[:], t_a[:], y_, OP.mult)      # v*y^2
            t_c = scr.tile([1, 1], DT, tag="nr_c", name="nr_c")
            nc.vector.tensor_scalar(t_c[:], t_b[:], -0.5, 1.5, OP.mult, OP.add)
            t_d = pers.tile([1, 1], DT, tag="nr_y" + str(it))
            nc.vector.tensor_tensor(t_d[:], t_c[:], y_, OP.mult)      # y'
            y_ = t_d[:]
        stdt = pers.tile([1, 1], DT, tag="stdt")
        nc.vector.tensor_tensor(stdt[:], vb[:], y_, OP.mult)          # sqrt(v)
        # broadcast to all partitions via PE: out[m,0] = ones_row[0,m] * stdt[0,0]
        ones_row = pers.tile([1, P], DT, tag="ones_row")
        nc.gpsimd.memset(ones_row[:], 1.0)
        ps_b = pp.tile([P, 1], DT, tag="ps_b")
        nc.tensor.matmul(ps_b[:], ones_row[:], stdt[:], start=True, stop=True)
        obb = pers.tile([P, 1], DT, tag="obb")
        nc.vector.tensor_copy(obb[:], ps_b[:])

        # ---------- re-layout c history to linear time order ----------
        clin = pers.tile([P, F], DT, tag="clin")
        clin_v = clin[:].rearrange("p (k i) -> p k i", i=L)
        ch_kl = ch[:].rearrange("p (s k) -> p k s", k=K)[:, :, W:W + L]
        nc.vector.tensor_copy(clin_v, ch_kl)

        # ---------- bulk elementwise outputs ----------
        def otile(nm, fdim=F):
            return pd.tile([P, fdim], DT, tag=nm, name=nm)

        sg2 = otile("sg2")
        if PHD: nc.scalar.activation(sg2[:], clin[:], AF.Sigmoid, bias=b_eoo[:], scale=c["A_OO"])
        ols = otile("ols")
        if PHD: nc.scalar.activation(ols[:], u2l_t[:], AF.Sigmoid, bias=b_bol[:], scale=c["SU2"])
        g_ol = otile("g_ol")
        if PHD: nc.scalar.activation(g_ol[:], ols[:], AF.Copy, bias=0.0, scale=c["OL1"])
        dd = otile("dd")
        if PHD: nc.vector.tensor_scalar(dd[:], u1l_t[:], c["D1"], c["D0"], OP.mult, OP.add)
        sarg = otile("sarg")
        if PHD: nc.vector.tensor_tensor(sarg[:], clin[:], dd[:], OP.add)
        g_ib = otile("g_ib")
        if PHD: nc.scalar.activation(g_ib[:], sarg[:], AF.Sigmoid, scale=c["A_IB"])
        g_oo = otile("g_oo")
        if PHD: nc.scalar.activation(g_oo[:], sg2[:], AF.Copy, bias=0.0, scale=c["OO1"])
        h0 = otile("h0")
        if PHD: nc.vector.tensor_tensor(h0[:], g_oo[:], clin[:], OP.mult)
        bp = otile("bp")
        if PHD: nc.vector.tensor_tensor(bp[:], g_ib[:], u1l_t[:], OP.mult)
        hn = otile("hn")
        if PHD: nc.vector.tensor_tensor(hn[:], h0[:], bp[:], OP.add)
        ln = otile("ln")
        if PHD: nc.gpsimd.tensor_tensor(ln[:], g_ol[:], clin[:], OP.mult)
        lcn = otile("lcn")
        if PHD: nc.vector.tensor_tensor(lcn[:], ln[:], u2l_t[:], OP.min)
        cg = otile("cg")
        rcp = otile("rcp")
        ur = otile("ur")
        g_olc = otile("g_olc")
        so_ = otile("so_")
        g_f = otile("g_f")
        H2 = F // 2
        for h in range(2):
            sl = slice(h * H2, (h + 1) * H2)
            if PHD:
                nc.vector.tensor_scalar(cg[:, sl], clin[:, sl], 1e-30, None, OP.max)
                nc.vector.reciprocal(rcp[:, sl], cg[:, sl])
                nc.gpsimd.tensor_tensor(ur[:, sl], u2l_t[:, sl], rcp[:, sl], OP.mult)
                nc.vector.tensor_tensor(g_olc[:, sl], g_ol[:, sl], ur[:, sl], OP.min)
                nc.gpsimd.tensor_tensor(so_[:, sl], g_oo[:, sl], g_olc[:, sl], OP.add)
                nc.scalar.activation(g_f[:, sl], so_[:, sl], AF.Copy, bias=1.0, scale=-1.0)

        onesF = pers.tile([P, F], DT, tag="onesF")
        nc.vector.memset(onesF[:], 1.0)
        obst = otile("obst")
        if PHD: nc.scalar.activation(obst[:], onesF[:], AF.Copy, bias=0.0, scale=obb[:])
        hio = otile("hio", 2 * F)
        hiov = hio[:].rearrange("p (f two) -> p f two", two=2)
        if PHD: nc.scalar.activation(hiov[:, :, 0], hn[:], AF.Copy, bias=0.0, scale=1.0)
        if PHD: nc.scalar.activation(hiov[:, :, 1], onesF[:], AF.Copy, bias=0.0, scale=obb[:])

        # ---------- outputs ----------
        if "outdma" in DBG_SKIP:
            nc.sync.dma_start(outs["o_c"][:], clin[:])
        else:
            for nm, t in [("o_c", clin), ("o_obs", obst), ("o_ol", g_ol), ("o_l", ln),
                          ("o_lc", lcn), ("o_ib", g_ib), ("o_oo", g_oo), ("o_bp", bp),
                          ("o_h", hn)]:
                nc.sync.dma_start(outs[nm][:], t[:])
            for h in range(2):
                sl = slice(h * H2, (h + 1) * H2)
                nc.sync.dma_start(outs["o_olc"][:, sl], g_olc[:, sl])
                nc.sync.dma_start(outs["o_f"][:, sl], g_f[:, sl])
                sl2 = slice(h * F, (h + 1) * F)
                nc.sync.dma_start(outs["o_hio"][:, sl2], hio[:, sl2])

    legalize_waits(nc)
    return nc


def _consts(inputs):
    mo = float(inputs["cmean"][0]); so = float(inputs["cstd"][0])
    e_o = math.exp(float(inputs["weight_r_yom"][0, 0]))
    e_l = math.exp(float(inputs["weight_r_ylm"][0, 0]))
    e_f = math.exp(float(inputs["weight_r_yfm"][0, 0]))
    den = e_o + e_l + e_f
    b0_yom = float(inputs["bias_b0_yom"][0]); w_b1_yom = float(inputs["weight_b1_yom"][0, 0])
    b0_ylm = float(inputs["bias_b0_ylm"][0]); w_b2_ylm = float(inputs["weight_b2_ylm"][0, 0])
    w_b1_yum = float(inputs["weight_b1_yum"][0, 0]); b0_yum = float(inputs["bias_b0_yum"][0])
    a_ib = w_b1_yum / so
    return {
        "OO1": e_o / den, "OL1": e_l / den,
        "A_IB": a_ib, "A_OO": w_b1_yom / so,
        "E_OO": b0_yom - w_b1_yom * mo / so,
        "SU2": w_b2_ylm / SL, "B_OL": b0_ylm - w_b2_ylm * ML / SL,
        "D0": (b0_yum - w_b1_yum * mo / so) / a_ib,
        "D1": (w_b1_yum / U1_MAX) / a_ib,
    }


def make_in_maps(inputs):
    x = np.asarray(inputs["x"], dtype=f32)
    y_obs = np.asarray(inputs["y_obs"], dtype=f32)
    u1 = np.ascontiguousarray(x[:, 0, 0])
    u2 = np.ascontiguousarray(x[:, 0, 1])
    GLEN = NCORE * PAD
    gp1 = np.zeros(W + GLEN, f32); gp1[W:W + B] = u1
    gp2 = np.zeros(W + GLEN, f32); gp2[W:W + B] = u2

    ys = np.zeros(P * YF, f32)
    ys[:NYO] = y_obs[SPIN:TRAIN, 0]
    ysq = ys.reshape(P, YF)

    jj = np.arange(P * K)                      # chunk within core (p*K + k)
    ii = np.arange(S)
    loc = jj[:, None] * L + ii[None, :]        # (PK, S); padded idx = base + loc
    in_maps = []
    for cid in range(NCORE):
        base = cid * PER_CORE
        g = gp1[base + loc]
        u1t = np.ascontiguousarray(g.reshape(P, K, S).transpose(0, 2, 1).reshape(P, S * K))
        g = gp2[base + loc]
        u2t = np.ascontiguousarray(g.reshape(P, K, S).transpose(0, 2, 1).reshape(P, S * K))
        u1lin = gp1[W + base: W + base + PAD].reshape(P, F)
        u2lin = gp2[W + base: W + base + PAD].reshape(P, F)
        in_maps.append({
            "u1t": u1t, "u2t": u2t,
            "u1l": np.ascontiguousarray(u1lin), "u2l": np.ascontiguousarray(u2lin),
            "yob": ysq,
        })
    return in_maps


def kernel(**inputs):
    consts = _consts(inputs)
    nc = build_program(consts)
    in_maps = make_in_maps(inputs)
    res = run_bass_kernel_spmd(nc, in_maps, list(range(NCORE)))
    results = res.results

    tl = int(np.asarray(inputs.get("time_lag", 0)))

    def gather(nm):
        return np.concatenate([results[cid][nm].reshape(-1)[:PER_CORE]
                               for cid in range(NCORE)])[:, None]

    h_n = gather("o_h"); c_n = gather("o_c"); l_n = gather("o_l"); lc_n = gather("o_lc")
    bp_n = gather("o_bp"); g_ib = gather("o_ib"); g_oo = gather("o_oo"); g_ol = gather("o_ol")
    g_olc = gather("o_olc"); g_f = gather("o_f"); obs_std = gather("o_obs")
    hio = np.concatenate([results[cid]["o_hio"].reshape(-1)[:2 * PER_CORE]
                          for cid in range(NCORE)]).reshape(B, 2)
    outs = [h_n, c_n, l_n, lc_n, bp_n, g_ib, g_oo, g_ol, g_olc, g_f, hio, obs_std]
    if tl > 0:
        for a in outs:
            a[:tl] = 0.0
    return tuple(np.ascontiguousarray(a, dtype=f32) for a in outs)


# revision 26
# speedup vs baseline: 1387.8237x; 1387.8237x over previous
"""Trainium2 Bass kernel for nn_MCPBRNN_Generic_PETconstraint_Scaling_BYPASSM1.

Algorithm
---------
The module is a 1M-step scalar (H=1) nonlinear recurrence
    c' = c - oo1*sig(A_OO*c+E_OO)*c - min(ol_t*c, u2_t) + (1-sig(A_IB*c+D_t))*u1_t
followed by 12 elementwise outputs.  The recurrence is strongly contracting
(|dc'/dc| in [0.37, 0.69] measured along the true trajectory; worst 24-step
window product ~3e-7), so the time axis can be chunked: each chunk starts from
c=0 and runs W warm-up steps over the preceding inputs, after which its
state agrees with the true trajectory to below f32 resolution.

Sharding: time axis split across 8 cores (125000 steps each); within a core,
128 partitions x K=64 slots = 8192 chunks of L=16 steps stepped in SIMD
lock-step.  Per step the work is split across engines: 2 sigmoids on ACT,
4 TT + 1 TS on DVE, and the min-branch (3 ops) on GpSimd.  The c history is
then re-laid out to linear time order with one strided tensor_copy and all
outputs are computed in bulk elementwise passes (ACT affine copies + DVE +
GpSimd) and DMA'd out contiguously.  obsstd = std(y_obs[1000:800000], ddof=1)
is accumulated by ACT Copy/Square accum_out chunks interleaved into the
scan's ACT idle slots, then finished with two PE matmuls and a Newton rsqrt.
"""
import math
import numpy as np
from contextlib import ExitStack

import concourse.bass as bass
import concourse.mybir as mybir
import concourse.tile as tile
from concourse.bass_utils import run_bass_kernel_spmd

f32 = np.float32

# ---- problem constants (hardcoded from the module definition) ----
B = 1000000
SPIN, TRAIN = 1000, 800000
ML, SL, U1_MAX = 2.9086, 1.898, 221.519

# ---- sharding geometry ----
NCORE = 8
P = 128          # SBUF partitions
K = 64           # chunk slots per partition
L = 16           # real steps per chunk
W = 24           # warm-up steps per chunk
S = W + L        # total scan steps (40)
F = K * L        # linear free size per partition (1024)
PAD = P * F      # padded per-core length (131072)
PER_CORE = B // NCORE   # 125000
NYO = TRAIN - SPIN      # 799000
YF = 6272               # yobs free size: 128*6272 = 802816 >= NYO
NOBS = 16               # obs accumulation chunks per statistic
YCH = YF // NOBS        # 392 elements per chunk
OBS_START = 12          # first scan step after which obs chunks interleave

DBG_SKIP = set()        # debug: subsets of {"scan", "phased", "obs", "outdma"}
REPEAT_SCAN = 1         # debug: repeat the scan loop to amplify timing
SCAN_VARIANT = "pool_m"  # "dve_all" | "pool_m" | "pool_mp"

DT = mybir.dt.float32
AF = mybir.ActivationFunctionType
OP = mybir.AluOpType


def legalize_waits(nc, max_waits=1):
    """This toolchain's walrus accepts only one sync-wait per compute
    instruction.  Hoist extra waits onto same-engine NOPs inserted right
    before the gated instruction (engine queues execute in order, so the
    semantics are identical)."""
    eng_map = {
        mybir.EngineType.DVE: nc.vector,
        mybir.EngineType.Activation: nc.scalar,
        mybir.EngineType.Pool: nc.gpsimd,
        mybir.EngineType.PE: nc.tensor,
        mybir.EngineType.SP: nc.sync,
    }
    blocks = nc.m.functions[0].blocks

    def detach(ins_obj):
        for bb2 in blocks:
            try:
                bb2.instructions.remove(ins_obj)
                return
            except ValueError:
                continue

    for bb in blocks:
        i = 0
        while i < len(bb.instructions):
            inst = bb.instructions[i]
            si = getattr(inst, "sync_info", None)
            if si is not None and si.on_wait and len(si.on_wait) > max_waits \
                    and inst.engine in eng_map:
                waits = list(si.on_wait)
                keep, extras = waits[-max_waits:], waits[:-max_waits]
                inst.sync_info = mybir.SyncInfo(on_wait=keep, on_update=list(si.on_update))
                e = eng_map[inst.engine]
                for w in extras:
                    nop = e.nop().ins
                    detach(nop)
                    nop.sync_info = mybir.SyncInfo(on_wait=[w], on_update=[])
                    bb.instructions.insert(i, nop)
                    i += 1
            i += 1


def build_program(c):
    """c: dict of baked float constants."""
    nc = bass.Bass()
    u1t = nc.declare_dram_parameter("u1t", [P, S * K], DT, isOutput=False)
    u2t = nc.declare_dram_parameter("u2t", [P, S * K], DT, isOutput=False)
    u1l = nc.declare_dram_parameter("u1l", [P, F], DT, isOutput=False)
    u2l = nc.declare_dram_parameter("u2l", [P, F], DT, isOutput=False)
    yob = nc.declare_dram_parameter("yob", [P, YF], DT, isOutput=False)
    outs = {}
    for nm in ["o_h", "o_c", "o_l", "o_lc", "o_bp", "o_ib", "o_oo", "o_ol", "o_olc", "o_f", "o_obs"]:
        outs[nm] = nc.declare_dram_parameter(nm, [P, F], DT, isOutput=True)
    outs["o_hio"] = nc.declare_dram_parameter("o_hio", [P, 2 * F], DT, isOutput=True)

    OBS = "obs" not in DBG_SKIP
    NSTEP = S if "scan" not in DBG_SKIP else 1
    PHD = "phased" not in DBG_SKIP

    with tile.TileContext(nc) as tc, ExitStack() as ctx:
        pers = ctx.enter_context(tc.tile_pool(name="pers", bufs=1))
        scr = ctx.enter_context(tc.tile_pool(name="scr", bufs=4))
        pd = ctx.enter_context(tc.tile_pool(name="pd", bufs=1))
        pp = ctx.enter_context(tc.tile_pool(name="psum", bufs=1, space="PSUM"))

        # ---------- load inputs (scan inputs in step-chunks for early start) ----------
        NCH = 5
        CHB = [0, 4, 10, 18, 28, S]            # step boundaries per chunk
        u1t_t = pers.tile([P, S * K], DT, tag="u1t_t")
        u2t_t = pers.tile([P, S * K], DT, tag="u2t_t")
        for q in range(NCH):
            lo, hi = CHB[q] * K, CHB[q + 1] * K
            nc.sync.dma_start(u1t_t[:, lo:hi], u1t[:, lo:hi])
            nc.sync.dma_start(u2t_t[:, lo:hi], u2t[:, lo:hi])
        u1l_t = pers.tile([P, F], DT, tag="u1l_t")
        u2l_t = pers.tile([P, F], DT, tag="u2l_t")
        nc.sync.dma_start(u1l_t[:], u1l[:])
        nc.sync.dma_start(u2l_t[:], u2l[:])
        yt = pers.tile([P, YF], DT, tag="yt")
        if OBS:
            nc.sync.dma_start(yt[:], yob[:])

        # bias tiles for activations (const-AP pool only has 0.0/1.0)
        b_bol = pers.tile([P, 1], DT, tag="b_bol")
        nc.vector.memset(b_bol[:], c["B_OL"])
        b_eoo = pers.tile([P, 1], DT, tag="b_eoo")
        nc.vector.memset(b_eoo[:], c["E_OO"])
        b_ibd = pers.tile([P, 1], DT, tag="b_ibd")
        nc.vector.memset(b_ibd[:], c["A_IB"] * c["D0"])

        # ---------- scan input prep (same chunking as the DMAs) ----------
        # Dp = D0 + D1*u1   (ib = sig(A_IB*(c + Dp)))
        dpt = pers.tile([P, S * K], DT, tag="dpt")
        olt = pers.tile([P, S * K], DT, tag="olt")
        for q in range(NCH):
            lo, hi = CHB[q] * K, CHB[q + 1] * K
            nc.vector.tensor_scalar(dpt[:, lo:hi], u1t_t[:, lo:hi], c["D1"], c["D0"], OP.mult, OP.add)
            # ol = OL1 * sig(SU2*u2 + B_OL)
            nc.scalar.activation(olt[:, lo:hi], u2t_t[:, lo:hi], AF.Sigmoid, bias=b_bol[:], scale=c["SU2"])
            nc.vector.tensor_scalar(olt[:, lo:hi], olt[:, lo:hi], c["OL1"], None, OP.mult)

        # obs sums: Sum(y) on idle PE; y^2 via ACT Square chunks interleaved
        # into the scan's ACT idle slots (in-place, after Sum(y) matmuls);
        # then Sum(y^2) on PE again.
        ones128 = pers.tile([P, 1], DT, tag="ones128")
        nc.gpsimd.memset(ones128[:], 1.0)
        ps_sy = pp.tile([P, 1], DT, tag="ps_sy")
        ps_sy2 = pp.tile([P, 1], DT, tag="ps_sy2")
        SQK = YF // 128
        obs_jobs = []
        if OBS:
            for q in range(SQK):
                obs_jobs.append(("my", q))
            for j in range(NOBS):
                obs_jobs.append(("sq", j))
            for q in range(SQK):
                obs_jobs.append(("my2", q))

        def emit_obs_job(job):
            kind, j = job
            if kind == "my":
                nc.tensor.matmul(ps_sy[:], yt[:, j * 128:(j + 1) * 128], ones128[:],
                                 start=(j == 0), stop=(j == SQK - 1))
            elif kind == "sq":
                sl = yt[:, j * YCH:(j + 1) * YCH]
                nc.scalar.activation(sl, sl, AF.Square)
            else:
                nc.tensor.matmul(ps_sy2[:], yt[:, j * 128:(j + 1) * 128], ones128[:],
                                 start=(j == 0), stop=(j == SQK - 1))

        # ---------- the scan ----------
        ch = pers.tile([P, (S + 1) * K], DT, tag="ch")
        chv = ch[:].rearrange("p (s k) -> p s k", k=K)
        nc.vector.memset(chv[:, 0, :], 0.0)
        u1v = u1t_t[:].rearrange("p (s k) -> p s k", k=K)
        u2v = u2t_t[:].rearrange("p (s k) -> p s k", k=K)
        dpv = dpt[:].rearrange("p (s k) -> p s k", k=K)
        olv = olt[:].rearrange("p (s k) -> p s k", k=K)
        obs_i = 0
        for _rep in range(REPEAT_SCAN):
          for i in range(NSTEP):
            cc = chv[:, i, :]
            # critical chain first: t1 -> a1 -> g_ -> p_ -> c1
            t1 = scr.tile([P, K], DT, tag="t1", name="t1")
            nc.scalar.activation(t1[:], cc, AF.Sigmoid, bias=b_eoo[:], scale=c["A_OO"])
            s_ = scr.tile([P, K], DT, tag="s_", name="s_")
            nc.vector.tensor_tensor(s_[:], cc, dpv[:, i, :], OP.add)
            t2c = scr.tile([P, K], DT, tag="t2c", name="t2c")
            nc.scalar.activation(t2c[:], s_[:], AF.Sigmoid, scale=-c["A_IB"])   # 1-ib
            me = nc.gpsimd if SCAN_VARIANT in ("pool_m", "pool_mp", "pool_mq", "pool_mq3") else nc.vector
            pe_ = nc.gpsimd if SCAN_VARIANT in ("pool_mp", "pool_mq3") else nc.vector
            m_ = scr.tile([P, K], DT, tag="m_", name="m_")
            me.tensor_tensor(m_[:], cc, olv[:, i, :], OP.mult)
            m2 = scr.tile([P, K], DT, tag="m2", name="m2")
            nc.vector.tensor_tensor(m2[:], m_[:], u2v[:, i, :], OP.min)
            a1 = scr.tile([P, K], DT, tag="a1", name="a1")
            nc.vector.tensor_scalar(a1[:], t1[:], -c["OO1"], 1.0, OP.mult, OP.add)
            g_ = scr.tile([P, K], DT, tag="g_", name="g_")
            nc.vector.tensor_tensor(g_[:], cc, a1[:], OP.mult)
            p_ = scr.tile([P, K], DT, tag="p_", name="p_")
            pe_.tensor_tensor(p_[:], g_[:], m2[:], OP.subtract)
            q_ = scr.tile([P, K], DT, tag="q_", name="q_")
            qe_ = nc.gpsimd if SCAN_VARIANT in ("pool_mq", "pool_mq3") else nc.vector
            qe_.tensor_tensor(q_[:], t2c[:], u1v[:, i, :], OP.mult)
            nc.vector.tensor_tensor(chv[:, i + 1, :], p_[:], q_[:], OP.add)
            if i >= OBS_START:
                budget = 6
                while obs_i < len(obs_jobs) and budget > 0:
                    kind = obs_jobs[obs_i][0]
                    emit_obs_job(obs_jobs[obs_i]); obs_i += 1
                    budget -= 6 if kind == "sq" else 1
        while obs_i < len(obs_jobs):
            emit_obs_job(obs_jobs[obs_i]); obs_i += 1

        # ---------- finish obsstd: cross-partition + newton rsqrt ----------
        sb_sy = pers.tile([P, 1], DT, tag="sb_sy")
        sb_sy2 = pers.tile([P, 1], DT, tag="sb_sy2")
        if OBS:
            nc.vector.tensor_copy(sb_sy[:], ps_sy[:])
            nc.vector.tensor_copy(sb_sy2[:], ps_sy2[:])
        else:
            nc.vector.memset(sb_sy[:], 0.5)
            nc.vector.memset(sb_sy2[:], 0.5)
        ps_t1 = pp.tile([1, 1], DT, tag="ps_t1")
        ps_t2 = pp.tile([1, 1], DT, tag="ps_t2")
        nc.tensor.matmul(ps_t1[:], sb_sy[:], ones128[:], start=True, stop=True)
        nc.tensor.matmul(ps_t2[:], sb_sy2[:], ones128[:], start=True, stop=True)
        s1b = pers.tile([1, 1], DT, tag="s1b")
        s2b = pers.tile([1, 1], DT, tag="s2b")
        nc.vector.tensor_copy(s1b[:], ps_t1[:])
        nc.vector.tensor_copy(s2b[:], ps_t2[:])
        # var = (S2 - S1^2/n) / (n-1)
        va = pers.tile([1, 1], DT, tag="va")
        nc.vector.tensor_tensor(va[:], s1b[:], s1b[:], OP.mult)
        vb = pers.tile([1, 1], DT, tag="vb")
        nc.vector.scalar_tensor_tensor(vb[:], va[:], -1.0 / NYO, s2b[:], OP.mult, OP.add)
        nc.vector.tensor_scalar(vb[:], vb[:], 1.0 / (NYO - 1), None, OP.mult)
        # std = vb * rsqrt(vb) via bit-trick seed + 3 Newton iterations
        vbi = vb[:].bitcast(mybir.dt.int32)
        shr = pers.tile([1, 1], mybir.dt.int32, tag="shr")
        nc.vector.tensor_scalar(shr[:], vbi, 1, None, OP.arith_shift_right)
        kmagic = pers.tile([1, 1], mybir.dt.int32, tag="kmagic")
        nc.vector.memset(kmagic[:], 0x5F3759DF)
        seed = pers.tile([1, 1], mybir.dt.int32, tag="seed")
        nc.vector.tensor_tensor(seed[:], kmagic[:], shr[:], OP.subtract)
        y_ = seed[:].bitcast(mybir.dt.float32)
        for it in range(3):
            t_a = scr.tile([1, 1], DT, tag="nr_a", name="nr_a")
            nc.vector.tensor_tensor(t_a[:], vb[:], y_, OP.mult)       # v*y
            t_b = scr.tile([1, 1], DT, tag="nr_b", name="nr_b")
            nc.vector.tensor_tensor(t_b[:], t_a[:], y_, OP.mult)      # v*y^2
            t_c = scr.tile([1, 1], DT, tag="nr_c", name="nr_c")
            nc.vector.tensor_scalar(t_c[:], t_b[:], -0.5, 1.5, OP.mult, OP.add)
            t_d = pers.tile([1, 1], DT, tag="nr_y" + str(it))
            nc.vector.tensor_tensor(t_d[:], t_c[:], y_, OP.mult)      # y'
            y_ = t_d[:]
        stdt = pers.tile([1, 1], DT, tag="stdt")
        nc.vector.tensor_tensor(stdt[:], vb[:], y_, OP.mult)          # sqrt(v)
        # broadcast to all partitions via PE: out[m,0] = ones_row[0,m] * stdt[0,0]
        ones_row = pers.tile([1, P], DT, tag="ones_row")
        nc.gpsimd.memset(ones_row[:], 1.0)
        ps_b = pp.tile([P, 1], DT, tag="ps_b")
        nc.tensor.matmul(ps_b[:], ones_row[:], stdt[:], start=True, stop=True)
        obb = pers.tile([P, 1], DT, tag="obb")
        nc.vector.tensor_copy(obb[:], ps_b[:])

        # ---------- re-layout c history to linear time order ----------
        clin = pers.tile([P, F], DT, tag="clin")
        clin_v = clin[:].rearrange("p (k i) -> p k i", i=L)
        ch_kl = ch[:].rearrange("p (s k) -> p k s", k=K)[:, :, W:W + L]
        nc.vector.tensor_copy(clin_v, ch_kl)

        # ---------- bulk elementwise outputs ----------
        def otile(nm, fdim=F):
            return pd.tile([P, fdim], DT, tag=nm, name=nm)

        sg2 = otile("sg2")
        if PHD: nc.scalar.activation(sg2[:], clin[:], AF.Sigmoid, bias=b_eoo[:], scale=c["A_OO"])
        ols = otile("ols")
        if PHD: nc.scalar.activation(ols[:], u2l_t[:], AF.Sigmoid, bias=b_bol[:], scale=c["SU2"])
        g_ol = otile("g_ol")
        if PHD: nc.scalar.activation(g_ol[:], ols[:], AF.Copy, bias=0.0, scale=c["OL1"])
        sarg = otile("sarg")
        if PHD: nc.vector.scalar_tensor_tensor(sarg[:], u1l_t[:], c["D1"], clin[:], OP.mult, OP.add)
        g_ib = otile("g_ib")
        if PHD: nc.scalar.activation(g_ib[:], sarg[:], AF.Sigmoid, bias=b_ibd[:], scale=c["A_IB"])
        g_oo = otile("g_oo")
        if PHD: nc.scalar.activation(g_oo[:], sg2[:], AF.Copy, bias=0.0, scale=c["OO1"])
        h0 = otile("h0")
        if PHD: nc.gpsimd.tensor_tensor(h0[:], g_oo[:], clin[:], OP.mult)
        bp = otile("bp")
        if PHD: nc.vector.tensor_tensor(bp[:], g_ib[:], u1l_t[:], OP.mult)
        hn = otile("hn")
        if PHD: nc.vector.tensor_tensor(hn[:], h0[:], bp[:], OP.add)
        ln = otile("ln")
        if PHD: nc.gpsimd.tensor_tensor(ln[:], g_ol[:], clin[:], OP.mult)
        lcn = otile("lcn")
        if PHD: nc.vector.tensor_tensor(lcn[:], ln[:], u2l_t[:], OP.min)
        cg = otile("cg")
        rcp = otile("rcp")
        ur = otile("ur")
        g_olc = otile("g_olc")
        so_ = otile("so_")
        g_f = otile("g_f")
        H2 = F // 2
        for h in range(2):
            sl = slice(h * H2, (h + 1) * H2)
            if PHD:
                nc.vector.tensor_scalar(cg[:, sl], clin[:, sl], 1e-30, None, OP.max)
                nc.vector.reciprocal(rcp[:, sl], cg[:, sl])
                nc.gpsimd.tensor_tensor(ur[:, sl], u2l_t[:, sl], rcp[:, sl], OP.mult)
                nc.vector.tensor_tensor(g_olc[:, sl], g_ol[:, sl], ur[:, sl], OP.min)
                nc.gpsimd.tensor_tensor(so_[:, sl], g_oo[:, sl], g_olc[:, sl], OP.add)
                nc.scalar.activation(g_f[:, sl], so_[:, sl], AF.Copy, bias=1.0, scale=-1.0)

        onesF = pers.tile([P, F], DT, tag="onesF")
        nc.vector.memset(onesF[:], 1.0)
        obst = otile("obst")
        if PHD: nc.scalar.activation(obst[:], onesF[:], AF.Copy, bias=0.0, scale=obb[:])
        hio = otile("hio", 2 * F)
        hiov = hio[:].rearrange("p (f two) -> p f two", two=2)
        if PHD: nc.scalar.activation(hiov[:, :, 0], hn[:], AF.Copy, bias=0.0, scale=1.0)
        if PHD: nc.scalar.activation(hiov[:, :, 1], onesF[:], AF.Copy, bias=0.0, scale=obb[:])

        # ---------- outputs ----------
        if "outdma" in DBG_SKIP:
            nc.sync.dma_start(outs["o_c"][:], clin[:])
        else:
            for nm, t in [("o_c", clin), ("o_obs", obst), ("o_ol", g_ol), ("o_l", ln),
                          ("o_lc", lcn), ("o_ib", g_ib), ("o_oo", g_oo), ("o_bp", bp),
                          ("o_h", hn)]:
                nc.sync.dma_start(outs[nm][:], t[:])
            for h in range(2):
                sl = slice(h * H2, (h + 1) * H2)
                nc.sync.dma_start(outs["o_olc"][:, sl], g_olc[:, sl])
                nc.sync.dma_start(outs["o_f"][:, sl], g_f[:, sl])
                sl2 = slice(h * F, (h + 1) * F)
                nc.sync.dma_start(outs["o_hio"][:, sl2], hio[:, sl2])

    legalize_waits(nc)
    return nc


def _consts(inputs):
    mo = float(inputs["cmean"][0]); so = float(inputs["cstd"][0])
    e_o = math.exp(float(inputs["weight_r_yom"][0, 0]))
    e_l = math.exp(float(inputs["weight_r_ylm"][0, 0]))
    e_f = math.exp(float(inputs["weight_r_yfm"][0, 0]))
    den = e_o + e_l + e_f
    b0_yom = float(inputs["bias_b0_yom"][0]); w_b1_yom = float(inputs["weight_b1_yom"][0, 0])
    b0_ylm = float(inputs["bias_b0_ylm"][0]); w_b2_ylm = float(inputs["weight_b2_ylm"][0, 0])
    w_b1_yum = float(inputs["weight_b1_yum"][0, 0]); b0_yum = float(inputs["bias_b0_yum"][0])
    a_ib = w_b1_yum / so
    return {
        "OO1": e_o / den, "OL1": e_l / den,
        "A_IB": a_ib, "A_OO": w_b1_yom / so,
        "E_OO": b0_yom - w_b1_yom * mo / so,
        "SU2": w_b2_ylm / SL, "B_OL": b0_ylm - w_b2_ylm * ML / SL,
        "D0": (b0_yum - w_b1_yum * mo / so) / a_ib,
        "D1": (w_b1_yum / U1_MAX) / a_ib,
    }


def make_in_maps(inputs):
    x = np.asarray(inputs["x"], dtype=f32)
    y_obs = np.asarray(inputs["y_obs"], dtype=f32)
    u1 = np.ascontiguousarray(x[:, 0, 0])
    u2 = np.ascontiguousarray(x[:, 0, 1])
    GLEN = NCORE * PAD
    gp1 = np.zeros(W + GLEN, f32); gp1[W:W + B] = u1
    gp2 = np.zeros(W + GLEN, f32); gp2[W:W + B] = u2

    ys = np.zeros(P * YF, f32)
    ys[:NYO] = y_obs[SPIN:TRAIN, 0]
    ysq = ys.reshape(P, YF)

    jj = np.arange(P * K)                      # chunk within core (p*K + k)
    ii = np.arange(S)
    loc = jj[:, None] * L + ii[None, :]        # (PK, S); padded idx = base + loc
    in_maps = []
    for cid in range(NCORE):
        base = cid * PER_CORE
        g = gp1[base + loc]
        u1t = np.ascontiguousarray(g.reshape(P, K, S).transpose(0, 2, 1).reshape(P, S * K))
        g = gp2[base + loc]
        u2t = np.ascontiguousarray(g.reshape(P, K, S).transpose(0, 2, 1).reshape(P, S * K))
        u1lin = gp1[W + base: W + base + PAD].reshape(P, F)
        u2lin = gp2[W + base: W + base + PAD].reshape(P, F)
        in_maps.append({
            "u1t": u1t, "u2t": u2t,
            "u1l": np.ascontiguousarray(u1lin), "u2l": np.ascontiguousarray(u2lin),
            "yob": ysq,
        })
    return in_maps


def kernel(**inputs):
    consts = _consts(inputs)
    nc = build_program(consts)
    in_maps = make_in_maps(inputs)
    res = run_bass_kernel_spmd(nc, in_maps, list(range(NCORE)))
    results = res.results

    tl = int(np.asarray(inputs.get("time_lag", 0)))

    def gather(nm):
        return np.concatenate([results[cid][nm].reshape(-1)[:PER_CORE]
                               for cid in range(NCORE)])[:, None]

    h_n = gather("o_h"); c_n = gather("o_c"); l_n = gather("o_l"); lc_n = gather("o_lc")
    bp_n = gather("o_bp"); g_ib = gather("o_ib"); g_oo = gather("o_oo"); g_ol = gather("o_ol")
    g_olc = gather("o_olc"); g_f = gather("o_f"); obs_std = gather("o_obs")
    hio = np.concatenate([results[cid]["o_hio"].reshape(-1)[:2 * PER_CORE]
                          for cid in range(NCORE)]).reshape(B, 2)
    outs = [h_n, c_n, l_n, lc_n, bp_n, g_ib, g_oo, g_ol, g_olc, g_f, hio, obs_std]
    if tl > 0:
        for a in outs:
            a[:tl] = 0.0
    return tuple(np.ascontiguousarray(a, dtype=f32) for a in outs)


# revision 27
# speedup vs baseline: 1437.6380x; 1.0359x over previous
"""Trainium2 Bass kernel for nn_MCPBRNN_Generic_PETconstraint_Scaling_BYPASSM1.

Algorithm
---------
The module is a 1M-step scalar (H=1) nonlinear recurrence
    c' = c - oo1*sig(A_OO*c+E_OO)*c - min(ol_t*c, u2_t) + (1-sig(A_IB*c+D_t))*u1_t
followed by 12 elementwise outputs.  The recurrence is strongly contracting
(|dc'/dc| in [0.37, 0.69] measured along the true trajectory; worst 24-step
window product ~3e-7), so the time axis can be chunked: each chunk starts from
c=0 and runs W warm-up steps over the preceding inputs, after which its
state agrees with the true trajectory to below f32 resolution.

Sharding: time axis split across 8 cores (125000 steps each); within a core,
128 partitions x K=64 slots = 8192 chunks of L=16 steps stepped in SIMD
lock-step.  Per step the work is split across engines: 2 sigmoids on ACT,
4 TT + 1 TS on DVE, and the min-branch (3 ops) on GpSimd.  The c history is
then re-laid out to linear time order with one strided tensor_copy and all
outputs are computed in bulk elementwise passes (ACT affine copies + DVE +
GpSimd) and DMA'd out contiguously.  obsstd = std(y_obs[1000:800000], ddof=1)
is accumulated by ACT Copy/Square accum_out chunks interleaved into the
scan's ACT idle slots, then finished with two PE matmuls and a Newton rsqrt.
"""
import math
import numpy as np
from contextlib import ExitStack

import concourse.bass as bass
import concourse.mybir as mybir
import concourse.tile as tile
from concourse.bass_utils import run_bass_kernel_spmd

f32 = np.float32

# ---- problem constants (hardcoded from the module definition) ----
B = 1000000
SPIN, TRAIN = 1000, 800000
ML, SL, U1_MAX = 2.9086, 1.898, 221.519

# ---- sharding geometry ----
NCORE = 8
P = 128          # SBUF partitions
K = 64           # chunk slots per partition
L = 16           # real steps per chunk
W = 22           # warm-up steps per chunk
S = W + L        # total scan steps (38)
F = K * L        # linear free size per partition (1024)
PAD = P * F      # padded per-core length (131072)
PER_CORE = B // NCORE   # 125000
NYO = TRAIN - SPIN      # 799000
YF = 6272               # yobs free size: 128*6272 = 802816 >= NYO
NOBS = 16               # obs accumulation chunks per statistic
YCH = YF // NOBS        # 392 elements per chunk
OBS_START = 12          # first scan step after which obs chunks interleave

DBG_SKIP = set()        # debug: subsets of {"scan", "phased", "obs", "outdma"}
REPEAT_SCAN = 1         # debug: repeat the scan loop to amplify timing
SCAN_VARIANT = "pool_m"  # "dve_all" | "pool_m" | "pool_mp"

DT = mybir.dt.float32
AF = mybir.ActivationFunctionType
OP = mybir.AluOpType


def legalize_waits(nc, max_waits=1):
    """This toolchain's walrus accepts only one sync-wait per compute
    instruction.  Hoist extra waits onto same-engine NOPs inserted right
    before the gated instruction (engine queues execute in order, so the
    semantics are identical)."""
    eng_map = {
        mybir.EngineType.DVE: nc.vector,
        mybir.EngineType.Activation: nc.scalar,
        mybir.EngineType.Pool: nc.gpsimd,
        mybir.EngineType.PE: nc.tensor,
        mybir.EngineType.SP: nc.sync,
    }
    blocks = nc.m.functions[0].blocks

    def detach(ins_obj):
        for bb2 in blocks:
            try:
                bb2.instructions.remove(ins_obj)
                return
            except ValueError:
                continue

    for bb in blocks:
        i = 0
        while i < len(bb.instructions):
            inst = bb.instructions[i]
            si = getattr(inst, "sync_info", None)
            if si is not None and si.on_wait and len(si.on_wait) > max_waits \
                    and inst.engine in eng_map:
                waits = list(si.on_wait)
                keep, extras = waits[-max_waits:], waits[:-max_waits]
                inst.sync_info = mybir.SyncInfo(on_wait=keep, on_update=list(si.on_update))
                e = eng_map[inst.engine]
                for w in extras:
                    nop = e.nop().ins
                    detach(nop)
                    nop.sync_info = mybir.SyncInfo(on_wait=[w], on_update=[])
                    bb.instructions.insert(i, nop)
                    i += 1
            i += 1


def build_program(c):
    """c: dict of baked float constants."""
    nc = bass.Bass()
    u1t = nc.declare_dram_parameter("u1t", [P, S * K], DT, isOutput=False)
    u2t = nc.declare_dram_parameter("u2t", [P, S * K], DT, isOutput=False)
    u1l = nc.declare_dram_parameter("u1l", [P, F], DT, isOutput=False)
    u2l = nc.declare_dram_parameter("u2l", [P, F], DT, isOutput=False)
    yob = nc.declare_dram_parameter("yob", [P, YF], DT, isOutput=False)
    outs = {}
    for nm in ["o_h", "o_c", "o_l", "o_lc", "o_bp", "o_ib", "o_oo", "o_ol", "o_olc", "o_f", "o_obs"]:
        outs[nm] = nc.declare_dram_parameter(nm, [P, F], DT, isOutput=True)
    outs["o_hio"] = nc.declare_dram_parameter("o_hio", [P, 2 * F], DT, isOutput=True)

    OBS = "obs" not in DBG_SKIP
    NSTEP = S if "scan" not in DBG_SKIP else 1
    PHD = "phased" not in DBG_SKIP

    with tile.TileContext(nc) as tc, ExitStack() as ctx:
        pers = ctx.enter_context(tc.tile_pool(name="pers", bufs=1))
        scr = ctx.enter_context(tc.tile_pool(name="scr", bufs=4))
        pd = ctx.enter_context(tc.tile_pool(name="pd", bufs=1))
        pp = ctx.enter_context(tc.tile_pool(name="psum", bufs=1, space="PSUM"))

        # ---------- load inputs (scan inputs in step-chunks for early start) ----------
        NCH = 5
        CHB = [0, 4, 10, 18, 28, S]            # step boundaries per chunk
        u1t_t = pers.tile([P, S * K], DT, tag="u1t_t")
        u2t_t = pers.tile([P, S * K], DT, tag="u2t_t")
        for q in range(NCH):
            lo, hi = CHB[q] * K, CHB[q + 1] * K
            nc.sync.dma_start(u1t_t[:, lo:hi], u1t[:, lo:hi])
            nc.sync.dma_start(u2t_t[:, lo:hi], u2t[:, lo:hi])
        u1l_t = pers.tile([P, F], DT, tag="u1l_t")
        u2l_t = pers.tile([P, F], DT, tag="u2l_t")
        nc.sync.dma_start(u1l_t[:], u1l[:])
        nc.sync.dma_start(u2l_t[:], u2l[:])
        yt = pers.tile([P, YF], DT, tag="yt")
        if OBS:
            nc.sync.dma_start(yt[:], yob[:])

        # bias tiles for activations (const-AP pool only has 0.0/1.0)
        b_bol = pers.tile([P, 1], DT, tag="b_bol")
        nc.vector.memset(b_bol[:], c["B_OL"])
        b_eoo = pers.tile([P, 1], DT, tag="b_eoo")
        nc.vector.memset(b_eoo[:], c["E_OO"])
        b_ibd = pers.tile([P, 1], DT, tag="b_ibd")
        nc.vector.memset(b_ibd[:], c["A_IB"] * c["D0"])

        # ---------- scan input prep (same chunking as the DMAs) ----------
        # Dp = D0 + D1*u1   (ib = sig(A_IB*(c + Dp)))
        dpt = pers.tile([P, S * K], DT, tag="dpt")
        olt = pers.tile([P, S * K], DT, tag="olt")
        for q in range(NCH):
            lo, hi = CHB[q] * K, CHB[q + 1] * K
            nc.vector.tensor_scalar(dpt[:, lo:hi], u1t_t[:, lo:hi], c["D1"], c["D0"], OP.mult, OP.add)
            # ol = OL1 * sig(SU2*u2 + B_OL)
            nc.scalar.activation(olt[:, lo:hi], u2t_t[:, lo:hi], AF.Sigmoid, bias=b_bol[:], scale=c["SU2"])
            nc.vector.tensor_scalar(olt[:, lo:hi], olt[:, lo:hi], c["OL1"], None, OP.mult)

        # obs sums: Sum(y) on idle PE; y^2 via ACT Square chunks interleaved
        # into the scan's ACT idle slots (in-place, after Sum(y) matmuls);
        # then Sum(y^2) on PE again.
        ones128 = pers.tile([P, 1], DT, tag="ones128")
        nc.gpsimd.memset(ones128[:], 1.0)
        ps_sy = pp.tile([P, 1], DT, tag="ps_sy")
        ps_sy2 = pp.tile([P, 1], DT, tag="ps_sy2")
        SQK = YF // 128
        obs_jobs = []
        if OBS:
            for q in range(SQK):
                obs_jobs.append(("my", q))
            for j in range(NOBS):
                obs_jobs.append(("sq", j))
            for q in range(SQK):
                obs_jobs.append(("my2", q))

        def emit_obs_job(job):
            kind, j = job
            if kind == "my":
                nc.tensor.matmul(ps_sy[:], yt[:, j * 128:(j + 1) * 128], ones128[:],
                                 start=(j == 0), stop=(j == SQK - 1))
            elif kind == "sq":
                sl = yt[:, j * YCH:(j + 1) * YCH]
                nc.scalar.activation(sl, sl, AF.Square)
            else:
                nc.tensor.matmul(ps_sy2[:], yt[:, j * 128:(j + 1) * 128], ones128[:],
                                 start=(j == 0), stop=(j == SQK - 1))

        # ---------- the scan ----------
        ch = pers.tile([P, (S + 1) * K], DT, tag="ch")
        chv = ch[:].rearrange("p (s k) -> p s k", k=K)
        nc.vector.memset(chv[:, 0, :], 0.0)
        u1v = u1t_t[:].rearrange("p (s k) -> p s k", k=K)
        u2v = u2t_t[:].rearrange("p (s k) -> p s k", k=K)
        dpv = dpt[:].rearrange("p (s k) -> p s k", k=K)
        olv = olt[:].rearrange("p (s k) -> p s k", k=K)
        obs_i = 0
        for _rep in range(REPEAT_SCAN):
          for i in range(NSTEP):
            cc = chv[:, i, :]
            # critical chain first: t1 -> a1 -> g_ -> p_ -> c1
            t1 = scr.tile([P, K], DT, tag="t1", name="t1")
            nc.scalar.activation(t1[:], cc, AF.Sigmoid, bias=b_eoo[:], scale=c["A_OO"])
            s_ = scr.tile([P, K], DT, tag="s_", name="s_")
            nc.vector.tensor_tensor(s_[:], cc, dpv[:, i, :], OP.add)
            t2c = scr.tile([P, K], DT, tag="t2c", name="t2c")
            nc.scalar.activation(t2c[:], s_[:], AF.Sigmoid, scale=-c["A_IB"])   # 1-ib
            me = nc.gpsimd if SCAN_VARIANT in ("pool_m", "pool_mp", "pool_mq", "pool_mq3") else nc.vector
            pe_ = nc.gpsimd if SCAN_VARIANT in ("pool_mp", "pool_mq3") else nc.vector
            m_ = scr.tile([P, K], DT, tag="m_", name="m_")
            me.tensor_tensor(m_[:], cc, olv[:, i, :], OP.mult)
            m2 = scr.tile([P, K], DT, tag="m2", name="m2")
            nc.vector.tensor_tensor(m2[:], m_[:], u2v[:, i, :], OP.min)
            a1 = scr.tile([P, K], DT, tag="a1", name="a1")
            nc.vector.tensor_scalar(a1[:], t1[:], -c["OO1"], 1.0, OP.mult, OP.add)
            g_ = scr.tile([P, K], DT, tag="g_", name="g_")
            nc.vector.tensor_tensor(g_[:], cc, a1[:], OP.mult)
            p_ = scr.tile([P, K], DT, tag="p_", name="p_")
            pe_.tensor_tensor(p_[:], g_[:], m2[:], OP.subtract)
            q_ = scr.tile([P, K], DT, tag="q_", name="q_")
            qe_ = nc.gpsimd if SCAN_VARIANT in ("pool_mq", "pool_mq3") else nc.vector
            qe_.tensor_tensor(q_[:], t2c[:], u1v[:, i, :], OP.mult)
            nc.vector.tensor_tensor(chv[:, i + 1, :], p_[:], q_[:], OP.add)
            if i >= OBS_START:
                budget = 6
                while obs_i < len(obs_jobs) and budget > 0:
                    kind = obs_jobs[obs_i][0]
                    emit_obs_job(obs_jobs[obs_i]); obs_i += 1
                    budget -= 6 if kind == "sq" else 1
        while obs_i < len(obs_jobs):
            emit_obs_job(obs_jobs[obs_i]); obs_i += 1

        # ---------- finish obsstd: cross-partition + newton rsqrt ----------
        sb_sy = pers.tile([P, 1], DT, tag="sb_sy")
        sb_sy2 = pers.tile([P, 1], DT, tag="sb_sy2")
        if OBS:
            nc.vector.tensor_copy(sb_sy[:], ps_sy[:])
            nc.vector.tensor_copy(sb_sy2[:], ps_sy2[:])
        else:
            nc.vector.memset(sb_sy[:], 0.5)
            nc.vector.memset(sb_sy2[:], 0.5)
        ps_t1 = pp.tile([1, 1], DT, tag="ps_t1")
        ps_t2 = pp.tile([1, 1], DT, tag="ps_t2")
        nc.tensor.matmul(ps_t1[:], sb_sy[:], ones128[:], start=True, stop=True)
        nc.tensor.matmul(ps_t2[:], sb_sy2[:], ones128[:], start=True, stop=True)
        s1b = pers.tile([1, 1], DT, tag="s1b")
        s2b = pers.tile([1, 1], DT, tag="s2b")
        nc.vector.tensor_copy(s1b[:], ps_t1[:])
        nc.vector.tensor_copy(s2b[:], ps_t2[:])
        # var = (S2 - S1^2/n) / (n-1)
        va = pers.tile([1, 1], DT, tag="va")
        nc.vector.tensor_tensor(va[:], s1b[:], s1b[:], OP.mult)
        vb = pers.tile([1, 1], DT, tag="vb")
        nc.vector.scalar_tensor_tensor(vb[:], va[:], -1.0 / NYO, s2b[:], OP.mult, OP.add)
        nc.vector.tensor_scalar(vb[:], vb[:], 1.0 / (NYO - 1), None, OP.mult)
        # std = vb * rsqrt(vb) via bit-trick seed + 3 Newton iterations
        vbi = vb[:].bitcast(mybir.dt.int32)
        shr = pers.tile([1, 1], mybir.dt.int32, tag="shr")
        nc.vector.tensor_scalar(shr[:], vbi, 1, None, OP.arith_shift_right)
        kmagic = pers.tile([1, 1], mybir.dt.int32, tag="kmagic")
        nc.vector.memset(kmagic[:], 0x5F3759DF)
        seed = pers.tile([1, 1], mybir.dt.int32, tag="seed")
        nc.vector.tensor_tensor(seed[:], kmagic[:], shr[:], OP.subtract)
        y_ = seed[:].bitcast(mybir.dt.float32)
        for it in range(3):
            t_a = scr.tile([1, 1], DT, tag="nr_a", name="nr_a")
            nc.vector.tensor_tensor(t_a[:], vb[:], y_, OP.mult)       # v*y
            t_b = scr.tile([1, 1], DT, tag="nr_b", name="nr_b")
            nc.vector.tensor_tensor(t_b[:], t_a[:], y_, OP.mult)      # v*y^2
            t_c = scr.tile([1, 1], DT, tag="nr_c", name="nr_c")
            nc.vector.tensor_scalar(t_c[:], t_b[:], -0.5, 1.5, OP.mult, OP.add)
            t_d = pers.tile([1, 1], DT, tag="nr_y" + str(it))
            nc.vector.tensor_tensor(t_d[:], t_c[:], y_, OP.mult)      # y'
            y_ = t_d[:]
        stdt = pers.tile([1, 1], DT, tag="stdt")
        nc.vector.tensor_tensor(stdt[:], vb[:], y_, OP.mult)          # sqrt(v)
        # broadcast to all partitions via PE: out[m,0] = ones_row[0,m] * stdt[0,0]
        ones_row = pers.tile([1, P], DT, tag="ones_row")
        nc.gpsimd.memset(ones_row[:], 1.0)
        ps_b = pp.tile([P, 1], DT, tag="ps_b")
        nc.tensor.matmul(ps_b[:], ones_row[:], stdt[:], start=True, stop=True)
        obb = pers.tile([P, 1], DT, tag="obb")
        nc.vector.tensor_copy(obb[:], ps_b[:])

        # ---------- re-layout c history to linear time order ----------
        clin = pers.tile([P, F], DT, tag="clin")
        clin_v = clin[:].rearrange("p (k i) -> p k i", i=L)
        ch_kl = ch[:].rearrange("p (s k) -> p k s", k=K)[:, :, W:W + L]
        nc.vector.tensor_copy(clin_v, ch_kl)

        # ---------- bulk elementwise outputs ----------
        def otile(nm, fdim=F):
            return pd.tile([P, fdim], DT, tag=nm, name=nm)

        sg2 = otile("sg2")
        if PHD: nc.scalar.activation(sg2[:], clin[:], AF.Sigmoid, bias=b_eoo[:], scale=c["A_OO"])
        ols = otile("ols")
        if PHD: nc.scalar.activation(ols[:], u2l_t[:], AF.Sigmoid, bias=b_bol[:], scale=c["SU2"])
        g_ol = otile("g_ol")
        if PHD: nc.scalar.activation(g_ol[:], ols[:], AF.Copy, bias=0.0, scale=c["OL1"])
        sarg = otile("sarg")
        if PHD: nc.vector.scalar_tensor_tensor(sarg[:], u1l_t[:], c["D1"], clin[:], OP.mult, OP.add)
        g_ib = otile("g_ib")
        if PHD: nc.scalar.activation(g_ib[:], sarg[:], AF.Sigmoid, bias=b_ibd[:], scale=c["A_IB"])
        g_oo = otile("g_oo")
        if PHD: nc.scalar.activation(g_oo[:], sg2[:], AF.Copy, bias=0.0, scale=c["OO1"])
        h0 = otile("h0")
        if PHD: nc.gpsimd.tensor_tensor(h0[:], g_oo[:], clin[:], OP.mult)
        bp = otile("bp")
        if PHD: nc.vector.tensor_tensor(bp[:], g_ib[:], u1l_t[:], OP.mult)
        hn = otile("hn")
        if PHD: nc.vector.tensor_tensor(hn[:], h0[:], bp[:], OP.add)
        ln = otile("ln")
        if PHD: nc.gpsimd.tensor_tensor(ln[:], g_ol[:], clin[:], OP.mult)
        lcn = otile("lcn")
        if PHD: nc.vector.tensor_tensor(lcn[:], ln[:], u2l_t[:], OP.min)
        cg = otile("cg")
        rcp = otile("rcp")
        ur = otile("ur")
        g_olc = otile("g_olc")
        so_ = otile("so_")
        g_f = otile("g_f")
        H2 = F // 2
        for h in range(2):
            sl = slice(h * H2, (h + 1) * H2)
            if PHD:
                nc.vector.tensor_scalar(cg[:, sl], clin[:, sl], 1e-30, None, OP.max)
                nc.vector.reciprocal(rcp[:, sl], cg[:, sl])
                nc.gpsimd.tensor_tensor(ur[:, sl], u2l_t[:, sl], rcp[:, sl], OP.mult)
                nc.vector.tensor_tensor(g_olc[:, sl], g_ol[:, sl], ur[:, sl], OP.min)
                nc.gpsimd.tensor_tensor(so_[:, sl], g_oo[:, sl], g_olc[:, sl], OP.add)
                nc.scalar.activation(g_f[:, sl], so_[:, sl], AF.Copy, bias=1.0, scale=-1.0)

        onesF = pers.tile([P, F], DT, tag="onesF")
        nc.vector.memset(onesF[:], 1.0)
        obst = otile("obst")
        if PHD: nc.scalar.activation(obst[:], onesF[:], AF.Copy, bias=0.0, scale=obb[:])
        hio = otile("hio", 2 * F)
        hiov = hio[:].rearrange("p (f two) -> p f two", two=2)
        if PHD: nc.scalar.activation(hiov[:, :, 0], hn[:], AF.Copy, bias=0.0, scale=1.0)
        if PHD: nc.scalar.activation(hiov[:, :, 1], onesF[:], AF.Copy, bias=0.0, scale=obb[:])

        # ---------- outputs ----------
        if "outdma" in DBG_SKIP:
            nc.sync.dma_start(outs["o_c"][:], clin[:])
        else:
            for nm, t in [("o_c", clin), ("o_obs", obst), ("o_ol", g_ol), ("o_l", ln),
                          ("o_lc", lcn), ("o_ib", g_ib), ("o_oo", g_oo), ("o_bp", bp),
                          ("o_h", hn)]:
                nc.sync.dma_start(outs[nm][:], t[:])
            for h in range(2):
                sl = slice(h * H2, (h + 1) * H2)
                nc.sync.dma_start(outs["o_olc"][:, sl], g_olc[:, sl])
                nc.sync.dma_start(outs["o_f"][:, sl], g_f[:, sl])
                sl2 = slice(h * F, (h + 1) * F)
                nc.sync.dma_start(outs["o_hio"][:, sl2], hio[:, sl2])

    legalize_waits(nc)
    return nc


def _consts(inputs):
    mo = float(inputs["cmean"][0]); so = float(inputs["cstd"][0])
    e_o = math.exp(float(inputs["weight_r_yom"][0, 0]))
    e_l = math.exp(float(inputs["weight_r_ylm"][0, 0]))
    e_f = math.exp(float(inputs["weight_r_yfm"][0, 0]))
    den = e_o + e_l + e_f
    b0_yom = float(inputs["bias_b0_yom"][0]); w_b1_yom = float(inputs["weight_b1_yom"][0, 0])
    b0_ylm = float(inputs["bias_b0_ylm"][0]); w_b2_ylm = float(inputs["weight_b2_ylm"][0, 0])
    w_b1_yum = float(inputs["weight_b1_yum"][0, 0]); b0_yum = float(inputs["bias_b0_yum"][0])
    a_ib = w_b1_yum / so
    return {
        "OO1": e_o / den, "OL1": e_l / den,
        "A_IB": a_ib, "A_OO": w_b1_yom / so,
        "E_OO": b0_yom - w_b1_yom * mo / so,
        "SU2": w_b2_ylm / SL, "B_OL": b0_ylm - w_b2_ylm * ML / SL,
        "D0": (b0_yum - w_b1_yum * mo / so) / a_ib,
        "D1": (w_b1_yum / U1_MAX) / a_ib,
    }


def make_in_maps(inputs):
    x = np.asarray(inputs["x"], dtype=f32)
    y_obs = np.asarray(inputs["y_obs"], dtype=f32)
    u1 = np.ascontiguousarray(x[:, 0, 0])
    u2 = np.ascontiguousarray(x[:, 0, 1])
    GLEN = NCORE * PAD
    gp1 = np.zeros(W + GLEN, f32); gp1[W:W + B] = u1
    gp2 = np.zeros(W + GLEN, f32); gp2[W:W + B] = u2

    ys = np.zeros(P * YF, f32)
    ys[:NYO] = y_obs[SPIN:TRAIN, 0]
    ysq = ys.reshape(P, YF)

    jj = np.arange(P * K)                      # chunk within core (p*K + k)
    ii = np.arange(S)
    loc = jj[:, None] * L + ii[None, :]        # (PK, S); padded idx = base + loc
    in_maps = []
    for cid in range(NCORE):
        base = cid * PER_CORE
        g = gp1[base + loc]
        u1t = np.ascontiguousarray(g.reshape(P, K, S).transpose(0, 2, 1).reshape(P, S * K))
        g = gp2[base + loc]
        u2t = np.ascontiguousarray(g.reshape(P, K, S).transpose(0, 2, 1).reshape(P, S * K))
        u1lin = gp1[W + base: W + base + PAD].reshape(P, F)
        u2lin = gp2[W + base: W + base + PAD].reshape(P, F)
        in_maps.append({
            "u1t": u1t, "u2t": u2t,
            "u1l": np.ascontiguousarray(u1lin), "u2l": np.ascontiguousarray(u2lin),
            "yob": ysq,
        })
    return in_maps


def kernel(**inputs):
    consts = _consts(inputs)
    nc = build_program(consts)
    in_maps = make_in_maps(inputs)
    res = run_bass_kernel_spmd(nc, in_maps, list(range(NCORE)))
    results = res.results

    tl = int(np.asarray(inputs.get("time_lag", 0)))

    def gather(nm):
        return np.concatenate([results[cid][nm].reshape(-1)[:PER_CORE]
                               for cid in range(NCORE)])[:, None]

    h_n = gather("o_h"); c_n = gather("o_c"); l_n = gather("o_l"); lc_n = gather("o_lc")
    bp_n = gather("o_bp"); g_ib = gather("o_ib"); g_oo = gather("o_oo"); g_ol = gather("o_ol")
    g_olc = gather("o_olc"); g_f = gather("o_f"); obs_std = gather("o_obs")
    hio = np.concatenate([results[cid]["o_hio"].reshape(-1)[:2 * PER_CORE]
                          for cid in range(NCORE)]).reshape(B, 2)
    outs = [h_n, c_n, l_n, lc_n, bp_n, g_ib, g_oo, g_ol, g_olc, g_f, hio, obs_std]
    if tl > 0:
        for a in outs:
            a[:tl] = 0.0
    return tuple(np.ascontiguousarray(a, dtype=f32) for a in outs)


# revision 29
# speedup vs baseline: 1441.3071x; 1.0026x over previous
"""Trainium2 Bass kernel for nn_MCPBRNN_Generic_PETconstraint_Scaling_BYPASSM1.

Algorithm
---------
The module is a 1M-step scalar (H=1) nonlinear recurrence
    c' = c - oo1*sig(A_OO*c+E_OO)*c - min(ol_t*c, u2_t) + (1-sig(A_IB*c+D_t))*u1_t
followed by 12 elementwise outputs.  The recurrence is strongly contracting
(|dc'/dc| in [0.37, 0.69] measured along the true trajectory; worst 24-step
window product ~3e-7), so the time axis can be chunked: each chunk starts from
c=0 and runs W warm-up steps over the preceding inputs, after which its
state agrees with the true trajectory to below f32 resolution.

Sharding: time axis split across 8 cores (125000 steps each); within a core,
128 partitions x K=64 slots = 8192 chunks of L=16 steps stepped in SIMD
lock-step.  Per step the work is split across engines: 2 sigmoids on ACT,
4 TT + 1 TS on DVE, and the min-branch (3 ops) on GpSimd.  The c history is
then re-laid out to linear time order with one strided tensor_copy and all
outputs are computed in bulk elementwise passes (ACT affine copies + DVE +
GpSimd) and DMA'd out contiguously.  obsstd = std(y_obs[1000:800000], ddof=1)
is accumulated by ACT Copy/Square accum_out chunks interleaved into the
scan's ACT idle slots, then finished with two PE matmuls and a Newton rsqrt.
"""
import math
import numpy as np
from contextlib import ExitStack

import concourse.bass as bass
import concourse.mybir as mybir
import concourse.tile as tile
from concourse.bass_utils import run_bass_kernel_spmd

f32 = np.float32

# ---- problem constants (hardcoded from the module definition) ----
B = 1000000
SPIN, TRAIN = 1000, 800000
ML, SL, U1_MAX = 2.9086, 1.898, 221.519

# ---- sharding geometry ----
NCORE = 8
P = 128          # SBUF partitions
K = 64           # chunk slots per partition
L = 16           # real steps per chunk
W = 22           # warm-up steps per chunk
S = W + L        # total scan steps (38)
F = K * L        # linear free size per partition (1024)
PAD = P * F      # padded per-core length (131072)
PER_CORE = B // NCORE   # 125000
NYO = TRAIN - SPIN      # 799000
YF = 6272               # yobs free size: 128*6272 = 802816 >= NYO
NOBS = 16               # obs accumulation chunks per statistic
YCH = YF // NOBS        # 392 elements per chunk
OBS_START = 12          # first scan step after which obs chunks interleave

DBG_SKIP = set()        # debug: subsets of {"scan", "phased", "obs", "outdma"}
REPEAT_SCAN = 1         # debug: repeat the scan loop to amplify timing
SCAN_VARIANT = "pool_m"  # "dve_all" | "pool_m" | "pool_mp"

DT = mybir.dt.float32
AF = mybir.ActivationFunctionType
OP = mybir.AluOpType


def legalize_waits(nc, max_waits=1):
    """This toolchain's walrus accepts only one sync-wait per compute
    instruction.  Hoist extra waits onto same-engine NOPs inserted right
    before the gated instruction (engine queues execute in order, so the
    semantics are identical)."""
    eng_map = {
        mybir.EngineType.DVE: nc.vector,
        mybir.EngineType.Activation: nc.scalar,
        mybir.EngineType.Pool: nc.gpsimd,
        mybir.EngineType.PE: nc.tensor,
        mybir.EngineType.SP: nc.sync,
    }
    blocks = nc.m.functions[0].blocks

    def detach(ins_obj):
        for bb2 in blocks:
            try:
                bb2.instructions.remove(ins_obj)
                return
            except ValueError:
                continue

    for bb in blocks:
        i = 0
        while i < len(bb.instructions):
            inst = bb.instructions[i]
            si = getattr(inst, "sync_info", None)
            if si is not None and si.on_wait and len(si.on_wait) > max_waits \
                    and inst.engine in eng_map:
                waits = list(si.on_wait)
                keep, extras = waits[-max_waits:], waits[:-max_waits]
                inst.sync_info = mybir.SyncInfo(on_wait=keep, on_update=list(si.on_update))
                e = eng_map[inst.engine]
                for w in extras:
                    nop = e.nop().ins
                    detach(nop)
                    nop.sync_info = mybir.SyncInfo(on_wait=[w], on_update=[])
                    bb.instructions.insert(i, nop)
                    i += 1
            i += 1


def build_program(c):
    """c: dict of baked float constants."""
    nc = bass.Bass()
    u1t = nc.declare_dram_parameter("u1t", [P, S * K], DT, isOutput=False)
    u2t = nc.declare_dram_parameter("u2t", [P, S * K], DT, isOutput=False)
    u1l = nc.declare_dram_parameter("u1l", [P, F], DT, isOutput=False)
    u2l = nc.declare_dram_parameter("u2l", [P, F], DT, isOutput=False)
    yob = nc.declare_dram_parameter("yob", [P, YF], DT, isOutput=False)
    outs = {}
    for nm in ["o_h", "o_c", "o_l", "o_lc", "o_bp", "o_ib", "o_oo", "o_ol", "o_olc", "o_f", "o_obs"]:
        outs[nm] = nc.declare_dram_parameter(nm, [P, F], DT, isOutput=True)
    outs["o_hio"] = nc.declare_dram_parameter("o_hio", [P, 2 * F], DT, isOutput=True)

    OBS = "obs" not in DBG_SKIP
    NSTEP = S if "scan" not in DBG_SKIP else 1
    PHD = "phased" not in DBG_SKIP

    with tile.TileContext(nc) as tc, ExitStack() as ctx:
        pers = ctx.enter_context(tc.tile_pool(name="pers", bufs=1))
        scr = ctx.enter_context(tc.tile_pool(name="scr", bufs=4))
        pd = ctx.enter_context(tc.tile_pool(name="pd", bufs=1))
        pp = ctx.enter_context(tc.tile_pool(name="psum", bufs=1, space="PSUM"))

        # ---------- load inputs (scan inputs in step-chunks for early start) ----------
        NCH = 5
        CHB = [0, 4, 10, 18, 28, S]            # step boundaries per chunk
        u1t_t = pers.tile([P, S * K], DT, tag="u1t_t")
        u2t_t = pers.tile([P, S * K], DT, tag="u2t_t")
        for q in range(NCH):
            lo, hi = CHB[q] * K, CHB[q + 1] * K
            nc.sync.dma_start(u1t_t[:, lo:hi], u1t[:, lo:hi])
            nc.sync.dma_start(u2t_t[:, lo:hi], u2t[:, lo:hi])
        u1l_t = pers.tile([P, F], DT, tag="u1l_t")
        u2l_t = pers.tile([P, F], DT, tag="u2l_t")
        nc.sync.dma_start(u1l_t[:], u1l[:])
        nc.sync.dma_start(u2l_t[:], u2l[:])
        yt = pers.tile([P, YF], DT, tag="yt")
        if OBS:
            nc.sync.dma_start(yt[:], yob[:])

        # bias tiles for activations (const-AP pool only has 0.0/1.0)
        b_bol = pers.tile([P, 1], DT, tag="b_bol")
        nc.vector.memset(b_bol[:], c["B_OL"])
        b_eoo = pers.tile([P, 1], DT, tag="b_eoo")
        nc.vector.memset(b_eoo[:], c["E_OO"])
        b_ibd = pers.tile([P, 1], DT, tag="b_ibd")
        nc.vector.memset(b_ibd[:], c["A_IB"] * c["D0"])

        # ---------- scan input prep (same chunking as the DMAs) ----------
        # Dp = D0 + D1*u1   (ib = sig(A_IB*(c + Dp)))
        dpt = pers.tile([P, S * K], DT, tag="dpt")
        olt = pers.tile([P, S * K], DT, tag="olt")
        for q in range(NCH):
            lo, hi = CHB[q] * K, CHB[q + 1] * K
            nc.vector.tensor_scalar(dpt[:, lo:hi], u1t_t[:, lo:hi], c["D1"], c["D0"], OP.mult, OP.add)
            # ol = OL1 * sig(SU2*u2 + B_OL)
            nc.scalar.activation(olt[:, lo:hi], u2t_t[:, lo:hi], AF.Sigmoid, bias=b_bol[:], scale=c["SU2"])
            nc.vector.tensor_scalar(olt[:, lo:hi], olt[:, lo:hi], c["OL1"], None, OP.mult)

        # obs sums: Sum(y) on idle PE; y^2 via ACT Square chunks interleaved
        # into the scan's ACT idle slots (in-place, after Sum(y) matmuls);
        # then Sum(y^2) on PE again.
        ones128 = pers.tile([P, 1], DT, tag="ones128")
        nc.gpsimd.memset(ones128[:], 1.0)
        ps_sy = pp.tile([P, 1], DT, tag="ps_sy")
        ps_sy2 = pp.tile([P, 1], DT, tag="ps_sy2")
        SQK = YF // 128
        obs_jobs = []
        if OBS:
            for q in range(SQK):
                obs_jobs.append(("my", q))
            for j in range(NOBS):
                obs_jobs.append(("sq", j))
            for q in range(SQK):
                obs_jobs.append(("my2", q))

        def emit_obs_job(job):
            kind, j = job
            if kind == "my":
                nc.tensor.matmul(ps_sy[:], yt[:, j * 128:(j + 1) * 128], ones128[:],
                                 start=(j == 0), stop=(j == SQK - 1))
            elif kind == "sq":
                sl = yt[:, j * YCH:(j + 1) * YCH]
                nc.scalar.activation(sl, sl, AF.Square)
            else:
                nc.tensor.matmul(ps_sy2[:], yt[:, j * 128:(j + 1) * 128], ones128[:],
                                 start=(j == 0), stop=(j == SQK - 1))

        # ---------- the scan ----------
        ch = pers.tile([P, (S + 1) * K], DT, tag="ch")
        chv = ch[:].rearrange("p (s k) -> p s k", k=K)
        nc.vector.memset(chv[:, 0, :], 0.0)
        u1v = u1t_t[:].rearrange("p (s k) -> p s k", k=K)
        u2v = u2t_t[:].rearrange("p (s k) -> p s k", k=K)
        dpv = dpt[:].rearrange("p (s k) -> p s k", k=K)
        olv = olt[:].rearrange("p (s k) -> p s k", k=K)
        obs_i = 0
        for _rep in range(REPEAT_SCAN):
          for i in range(NSTEP):
            cc = chv[:, i, :]
            # critical chain first: t1 -> a1 -> g_ -> p_ -> c1
            t1 = scr.tile([P, K], DT, tag="t1", name="t1")
            nc.scalar.activation(t1[:], cc, AF.Sigmoid, bias=b_eoo[:], scale=c["A_OO"])
            s_ = scr.tile([P, K], DT, tag="s_", name="s_")
            se_ = nc.gpsimd if SCAN_VARIANT in ("pool_ms", "pool_msq") else nc.vector
            se_.tensor_tensor(s_[:], cc, dpv[:, i, :], OP.add)
            t2c = scr.tile([P, K], DT, tag="t2c", name="t2c")
            nc.scalar.activation(t2c[:], s_[:], AF.Sigmoid, scale=-c["A_IB"])   # 1-ib
            me = nc.gpsimd if SCAN_VARIANT in ("pool_m", "pool_mp", "pool_mq", "pool_mq3", "pool_ms", "pool_msq") else nc.vector
            pe_ = nc.gpsimd if SCAN_VARIANT in ("pool_mp", "pool_mq3") else nc.vector
            m_ = scr.tile([P, K], DT, tag="m_", name="m_")
            me.tensor_tensor(m_[:], cc, olv[:, i, :], OP.mult)
            m2 = scr.tile([P, K], DT, tag="m2", name="m2")
            nc.vector.tensor_tensor(m2[:], m_[:], u2v[:, i, :], OP.min)
            a1 = scr.tile([P, K], DT, tag="a1", name="a1")
            nc.vector.tensor_scalar(a1[:], t1[:], -c["OO1"], 1.0, OP.mult, OP.add)
            g_ = scr.tile([P, K], DT, tag="g_", name="g_")
            nc.vector.tensor_tensor(g_[:], cc, a1[:], OP.mult)
            p_ = scr.tile([P, K], DT, tag="p_", name="p_")
            pe_.tensor_tensor(p_[:], g_[:], m2[:], OP.subtract)
            q_ = scr.tile([P, K], DT, tag="q_", name="q_")
            qe_ = nc.gpsimd if SCAN_VARIANT in ("pool_mq", "pool_mq3", "pool_msq") else nc.vector
            qe_.tensor_tensor(q_[:], t2c[:], u1v[:, i, :], OP.mult)
            nc.vector.tensor_tensor(chv[:, i + 1, :], p_[:], q_[:], OP.add)
            if i >= OBS_START:
                budget = 6
                while obs_i < len(obs_jobs) and budget > 0:
                    kind = obs_jobs[obs_i][0]
                    emit_obs_job(obs_jobs[obs_i]); obs_i += 1
                    budget -= 6 if kind == "sq" else 1
        while obs_i < len(obs_jobs):
            emit_obs_job(obs_jobs[obs_i]); obs_i += 1

        # ---------- finish obsstd: cross-partition + newton rsqrt ----------
        sb_sy = pers.tile([P, 1], DT, tag="sb_sy")
        sb_sy2 = pers.tile([P, 1], DT, tag="sb_sy2")
        if OBS:
            nc.vector.tensor_copy(sb_sy[:], ps_sy[:])
            nc.vector.tensor_copy(sb_sy2[:], ps_sy2[:])
        else:
            nc.vector.memset(sb_sy[:], 0.5)
            nc.vector.memset(sb_sy2[:], 0.5)
        ps_t1 = pp.tile([1, 1], DT, tag="ps_t1")
        ps_t2 = pp.tile([1, 1], DT, tag="ps_t2")
        nc.tensor.matmul(ps_t1[:], sb_sy[:], ones128[:], start=True, stop=True)
        nc.tensor.matmul(ps_t2[:], sb_sy2[:], ones128[:], start=True, stop=True)
        s1b = pers.tile([1, 1], DT, tag="s1b")
        s2b = pers.tile([1, 1], DT, tag="s2b")
        nc.vector.tensor_copy(s1b[:], ps_t1[:])
        nc.vector.tensor_copy(s2b[:], ps_t2[:])
        # var = (S2 - S1^2/n) / (n-1)
        va = pers.tile([1, 1], DT, tag="va")
        nc.vector.tensor_tensor(va[:], s1b[:], s1b[:], OP.mult)
        vb = pers.tile([1, 1], DT, tag="vb")
        nc.vector.scalar_tensor_tensor(vb[:], va[:], -1.0 / NYO, s2b[:], OP.mult, OP.add)
        nc.vector.tensor_scalar(vb[:], vb[:], 1.0 / (NYO - 1), None, OP.mult)
        # std = vb * rsqrt(vb) via bit-trick seed + 3 Newton iterations
        vbi = vb[:].bitcast(mybir.dt.int32)
        shr = pers.tile([1, 1], mybir.dt.int32, tag="shr")
        nc.vector.tensor_scalar(shr[:], vbi, 1, None, OP.arith_shift_right)
        kmagic = pers.tile([1, 1], mybir.dt.int32, tag="kmagic")
        nc.vector.memset(kmagic[:], 0x5F3759DF)
        seed = pers.tile([1, 1], mybir.dt.int32, tag="seed")
        nc.vector.tensor_tensor(seed[:], kmagic[:], shr[:], OP.subtract)
        y_ = seed[:].bitcast(mybir.dt.float32)
        for it in range(3):
            t_a = scr.tile([1, 1], DT, tag="nr_a", name="nr_a")
            nc.vector.tensor_tensor(t_a[:], vb[:], y_, OP.mult)       # v*y
            t_b = scr.tile([1, 1], DT, tag="nr_b", name="nr_b")
            nc.vector.tensor_tensor(t_b[:], t_a[:], y_, OP.mult)      # v*y^2
            t_c = scr.tile([1, 1], DT, tag="nr_c", name="nr_c")
            nc.vector.tensor_scalar(t_c[:], t_b[:], -0.5, 1.5, OP.mult, OP.add)
            t_d = pers.tile([1, 1], DT, tag="nr_y" + str(it))
            nc.vector.tensor_tensor(t_d[:], t_c[:], y_, OP.mult)      # y'
            y_ = t_d[:]
        stdt = pers.tile([1, 1], DT, tag="stdt")
        nc.vector.tensor_tensor(stdt[:], vb[:], y_, OP.mult)          # sqrt(v)
        # broadcast to all partitions via PE: out[m,0] = ones_row[0,m] * stdt[0,0]
        ones_row = pers.tile([1, P], DT, tag="ones_row")
        nc.gpsimd.memset(ones_row[:], 1.0)
        ps_b = pp.tile([P, 1], DT, tag="ps_b")
        nc.tensor.matmul(ps_b[:], ones_row[:], stdt[:], start=True, stop=True)
        obb = pers.tile([P, 1], DT, tag="obb")
        nc.vector.tensor_copy(obb[:], ps_b[:])

        # ---------- re-layout c history to linear time order ----------
        clin = pers.tile([P, F], DT, tag="clin")
        clin_v = clin[:].rearrange("p (k i) -> p k i", i=L)
        ch_kl = ch[:].rearrange("p (s k) -> p k s", k=K)[:, :, W:W + L]
        nc.vector.tensor_copy(clin_v, ch_kl)

        # ---------- bulk elementwise outputs ----------
        def otile(nm, fdim=F):
            return pd.tile([P, fdim], DT, tag=nm, name=nm)

        sg2 = otile("sg2")
        if PHD: nc.scalar.activation(sg2[:], clin[:], AF.Sigmoid, bias=b_eoo[:], scale=c["A_OO"])
        ols = otile("ols")
        if PHD: nc.scalar.activation(ols[:], u2l_t[:], AF.Sigmoid, bias=b_bol[:], scale=c["SU2"])
        g_ol = otile("g_ol")
        if PHD: nc.scalar.activation(g_ol[:], ols[:], AF.Copy, bias=0.0, scale=c["OL1"])
        sarg = otile("sarg")
        if PHD: nc.vector.scalar_tensor_tensor(sarg[:], u1l_t[:], c["D1"], clin[:], OP.mult, OP.add)
        g_ib = otile("g_ib")
        if PHD: nc.scalar.activation(g_ib[:], sarg[:], AF.Sigmoid, bias=b_ibd[:], scale=c["A_IB"])
        g_oo = otile("g_oo")
        if PHD: nc.scalar.activation(g_oo[:], sg2[:], AF.Copy, bias=0.0, scale=c["OO1"])
        h0 = otile("h0")
        if PHD: nc.gpsimd.tensor_tensor(h0[:], g_oo[:], clin[:], OP.mult)
        bp = otile("bp")
        if PHD: nc.vector.tensor_tensor(bp[:], g_ib[:], u1l_t[:], OP.mult)
        hn = otile("hn")
        if PHD: nc.vector.tensor_tensor(hn[:], h0[:], bp[:], OP.add)
        ln = otile("ln")
        if PHD: nc.gpsimd.tensor_tensor(ln[:], g_ol[:], clin[:], OP.mult)
        lcn = otile("lcn")
        if PHD: nc.vector.tensor_tensor(lcn[:], ln[:], u2l_t[:], OP.min)
        cg = otile("cg")
        rcp = otile("rcp")
        ur = otile("ur")
        g_olc = otile("g_olc")
        so_ = otile("so_")
        g_f = otile("g_f")
        H2 = F // 4
        for h in range(4):
            sl = slice(h * H2, (h + 1) * H2)
            if PHD:
                nc.vector.tensor_scalar(cg[:, sl], clin[:, sl], 1e-30, None, OP.max)
                nc.vector.reciprocal(rcp[:, sl], cg[:, sl])
                nc.gpsimd.tensor_tensor(ur[:, sl], u2l_t[:, sl], rcp[:, sl], OP.mult)
                nc.vector.tensor_tensor(g_olc[:, sl], g_ol[:, sl], ur[:, sl], OP.min)
                nc.gpsimd.tensor_tensor(so_[:, sl], g_oo[:, sl], g_olc[:, sl], OP.add)
                nc.scalar.activation(g_f[:, sl], so_[:, sl], AF.Copy, bias=1.0, scale=-1.0)

        onesF = pers.tile([P, F], DT, tag="onesF")
        nc.vector.memset(onesF[:], 1.0)
        obst = otile("obst")
        if PHD: nc.scalar.activation(obst[:], onesF[:], AF.Copy, bias=0.0, scale=obb[:])
        hio = otile("hio", 2 * F)
        hiov = hio[:].rearrange("p (f two) -> p f two", two=2)
        if PHD: nc.scalar.activation(hiov[:, :, 0], hn[:], AF.Copy, bias=0.0, scale=1.0)
        if PHD: nc.scalar.activation(hiov[:, :, 1], onesF[:], AF.Copy, bias=0.0, scale=obb[:])

        # ---------- outputs ----------
        if "outdma" in DBG_SKIP:
            nc.sync.dma_start(outs["o_c"][:], clin[:])
        else:
            for nm, t in [("o_c", clin), ("o_obs", obst), ("o_ol", g_ol), ("o_l", ln),
                          ("o_lc", lcn), ("o_ib", g_ib), ("o_oo", g_oo), ("o_bp", bp),
                          ("o_h", hn)]:
                nc.sync.dma_start(outs[nm][:], t[:])
            for h in range(4):
                sl = slice(h * H2, (h + 1) * H2)
                nc.sync.dma_start(outs["o_olc"][:, sl], g_olc[:, sl])
                nc.sync.dma_start(outs["o_f"][:, sl], g_f[:, sl])
            for h in range(2):
                sl2 = slice(h * F, (h + 1) * F)
                nc.sync.dma_start(outs["o_hio"][:, sl2], hio[:, sl2])

    legalize_waits(nc)
    return nc


def _consts(inputs):
    mo = float(inputs["cmean"][0]); so = float(inputs["cstd"][0])
    e_o = math.exp(float(inputs["weight_r_yom"][0, 0]))
    e_l = math.exp(float(inputs["weight_r_ylm"][0, 0]))
    e_f = math.exp(float(inputs["weight_r_yfm"][0, 0]))
    den = e_o + e_l + e_f
    b0_yom = float(inputs["bias_b0_yom"][0]); w_b1_yom = float(inputs["weight_b1_yom"][0, 0])
    b0_ylm = float(inputs["bias_b0_ylm"][0]); w_b2_ylm = float(inputs["weight_b2_ylm"][0, 0])
    w_b1_yum = float(inputs["weight_b1_yum"][0, 0]); b0_yum = float(inputs["bias_b0_yum"][0])
    a_ib = w_b1_yum / so
    return {
        "OO1": e_o / den, "OL1": e_l / den,
        "A_IB": a_ib, "A_OO": w_b1_yom / so,
        "E_OO": b0_yom - w_b1_yom * mo / so,
        "SU2": w_b2_ylm / SL, "B_OL": b0_ylm - w_b2_ylm * ML / SL,
        "D0": (b0_yum - w_b1_yum * mo / so) / a_ib,
        "D1": (w_b1_yum / U1_MAX) / a_ib,
    }


def make_in_maps(inputs):
    x = np.asarray(inputs["x"], dtype=f32)
    y_obs = np.asarray(inputs["y_obs"], dtype=f32)
    u1 = np.ascontiguousarray(x[:, 0, 0])
    u2 = np.ascontiguousarray(x[:, 0, 1])
    GLEN = NCORE * PAD
    gp1 = np.zeros(W + GLEN, f32); gp1[W:W + B] = u1
    gp2 = np.zeros(W + GLEN, f32); gp2[W:W + B] = u2

    ys = np.zeros(P * YF, f32)
    ys[:NYO] = y_obs[SPIN:TRAIN, 0]
    ysq = ys.reshape(P, YF)

    jj = np.arange(P * K)                      # chunk within core (p*K + k)
    ii = np.arange(S)
    loc = jj[:, None] * L + ii[None, :]        # (PK, S); padded idx = base + loc
    in_maps = []
    for cid in range(NCORE):
        base = cid * PER_CORE
        g = gp1[base + loc]
        u1t = np.ascontiguousarray(g.reshape(P, K, S).transpose(0, 2, 1).reshape(P, S * K))
        g = gp2[base + loc]
        u2t = np.ascontiguousarray(g.reshape(P, K, S).transpose(0, 2, 1).reshape(P, S * K))
        u1lin = gp1[W + base: W + base + PAD].reshape(P, F)
        u2lin = gp2[W + base: W + base + PAD].reshape(P, F)
        in_maps.append({
            "u1t": u1t, "u2t": u2t,
            "u1l": np.ascontiguousarray(u1lin), "u2l": np.ascontiguousarray(u2lin),
            "yob": ysq,
        })
    return in_maps


def kernel(**inputs):
    consts = _consts(inputs)
    nc = build_program(consts)
    in_maps = make_in_maps(inputs)
    res = run_bass_kernel_spmd(nc, in_maps, list(range(NCORE)))
    results = res.results

    tl = int(np.asarray(inputs.get("time_lag", 0)))

    def gather(nm):
        return np.concatenate([results[cid][nm].reshape(-1)[:PER_CORE]
                               for cid in range(NCORE)])[:, None]

    h_n = gather("o_h"); c_n = gather("o_c"); l_n = gather("o_l"); lc_n = gather("o_lc")
    bp_n = gather("o_bp"); g_ib = gather("o_ib"); g_oo = gather("o_oo"); g_ol = gather("o_ol")
    g_olc = gather("o_olc"); g_f = gather("o_f"); obs_std = gather("o_obs")
    hio = np.concatenate([results[cid]["o_hio"].reshape(-1)[:2 * PER_CORE]
                          for cid in range(NCORE)]).reshape(B, 2)
    outs = [h_n, c_n, l_n, lc_n, bp_n, g_ib, g_oo, g_ol, g_olc, g_f, hio, obs_std]
    if tl > 0:
        for a in outs:
            a[:tl] = 0.0
    return tuple(np.ascontiguousarray(a, dtype=f32) for a in outs)
